# revision 19
# baseline (speedup 1.0000x reference)
"""ApproxNDCGLoss on 8 TRN2 NeuronCores (Bass/Tile) — one-collective design.

loss = 1 - dcg/(idcg+1e-8):
  approx_rank[j] = 1 + sum_i sigmoid(s[j]-s[i])
  dcg  = sum_j y[j] / log2(approx_rank[j]+1)
  idcg = sum_j y[j] / log2(rank_y[j]+1),  rank_y[j] = 1 + #{i: y[i] > y[j]}

Everything per-item is folded into per-BUCKET sums that are additive across
cores, so a single fused AllReduce replaces the old (T-table AR + per-item
lookup + partials AR) pipeline:

DCG:  sigmoid(x) - 1/2 ~= sum_k b_k sin(w_k x)  (K=32 sine series), so
  rank(t)+1 = n/2 + 2 + sum_k b_k [sin(w_k t) C_k - cos(w_k t) S_k]
  with C/S = global trig sums.  Scores are binned into 2048 buckets
  (64x32 two-level); per-bucket y-sums Ys are AllReduced, and post-AR the
  series is evaluated at all bucket centers with ONE 64-contraction matmul
  via the angle-addition split  theta = A(c1) + B(c2):
      ser[c1,c2] = sum_k U_k(c1) cosB_k(c2) + V_k(c1) sinB_k(c2)
  where U,V are [32,64] tiles built from (C,S) and host trig constants.
  dcg = sum_b Ys_b * ln2/ln(ser_b + n/2 + 2).  Bucketing error ~1e-6.

IDCG: y in [0,1) binned into 4096 buckets (64x64).  Joint histogram via
  one-hot matmuls; suffix-count table T (strict suffix + hist/2) is linear
  in hist so each core builds its local T pre-collective.  Per-bucket
  y-sums Ysum give  idcg = sum_b Ysum_b * ln2/ln(T_b + 1.5).

The collectives share DMA bandwidth with input loads across all 8 cores,
so the mesh cannot start until the aggregate input traffic drains — input
bytes are the critical resource.  Only ~325KB/core is shipped: compact
per-block ids/weights [20,128], raw quad scores [4,640], and small trig/
triangle constants.  On device, PE outer-product matmuls against an
iota-built block-selector expand ids/weights to one-hot compare operands
([20,128] @ [20,NB*64] -> PSUM), and w_k*s is produced by a 4-contraction
matmul against an omega-selector; big contiguous IS_EQ/MULT ops then build
the bf16 one-hot matmul operands.  No partition broadcasts, no per-item
post-AR work, and every DMA descriptor is fat.
"""

import numpy as np
import ml_dtypes

import concourse.bacc as bacc
import concourse.bass as bass
import concourse.mybir as mybir
import concourse.tile as tile
from concourse.bass_utils import run_bass_kernel_spmd
from concourse.tile_rust import add_dep_helper

N = 20000
NCORES = 4
PB = 5120                   # items per core (padded; 4*5120 = 20480)
NB = PB // 128              # column blocks of 128 items
QB = PB // 4                # free elems in the quad trig layout
HQ = 320                    # trig processed in 320-wide PSUM chunks
K = 32                      # Fourier terms
L = 24.2                    # period of the sine series
TRIG_PAD = NCORES * PB - N  # 480 zero-score pads -> C_k -= 480
# y buckets: 4096 = 64 (partitions) x 64 (free)
QSY = 4096
C2Y = 64
W64 = NB * C2Y              # 1280
# score buckets: 2048 = 64 (partitions) x 32 (free)
MBS = 2048
C2S = 32
W32 = NB * C2S              # 640
LO, HI = -5.5, 5.5
DELTA = (HI - LO) / MBS
LN2 = float(np.log(2.0))

_B = np.array([
    0.575840175151825, -0.0012469458160921931, 0.08171718567609787,
    0.019092485308647156, -0.007231124211102724, 0.02490580640733242,
    -0.017197489738464355, 0.014312449842691422, -0.007428332697600126,
    0.003442077897489071, -0.0007101596565917134, 3.444465983193368e-05,
    -0.00029458850622177124, 0.0009411321370862424, -0.0013493510195985436,
    0.0013473577564582229, -0.0009938474977388978, 0.0005221660248935223,
    -0.00015226299001369625, 2.9422192255879054e-06, -5.903289275011048e-05,
    0.00021578818268608302, -0.0003499265294522047, 0.0003830934874713421,
    -0.00030826698639430106, 0.0001763014297466725, -5.747509567299858e-05,
    2.007998773478903e-06, -1.8746375644695945e-05, 7.875602022977546e-05,
    -0.00013714544184040278, 0.00015883310697972775], dtype=np.float32)
_OMEGA = (2.0 * np.pi * np.arange(1, K + 1) / L).astype(np.float32)

# range reduction: m = x - round(x/2pi)*2pi via magic-number round and a
# 3-term Cody-Waite cascade.
_MAGIC = float(np.float32(1.5 * 2.0 ** 23))
_INV2PI = float(np.float32(1.0 / (2.0 * np.pi)))
_CW1 = 6.28125
_CW2 = float(np.float32(2.0 * np.pi - 6.28125))
_CW3 = float(np.float32(2.0 * np.pi - 6.28125
                        - np.float64(np.float32(2.0 * np.pi - 6.28125))))
_PI = float(np.pi)

_CACHE = {}


def _build():
    f32 = mybir.dt.float32
    bf16 = mybir.dt.bfloat16
    AF = mybir.ActivationFunctionType
    ALU = mybir.AluOpType
    X = mybir.AxisListType.X

    nc = bacc.Bacc("TRN2", target_bir_lowering=False, debug=False,
                   num_devices=NCORES)
    # qT rows: per-block lhsT data [NB, 128] each: q1y | q2y | qs1 | qs2 | y
    qt_dram = nc.dram_tensor("qT", [NB, 5 * 128], bf16,
                             kind="ExternalInput")
    # s_row2: quad scores [4, 640] | omega-selector wsel [4, 128]
    sr_dram = nc.dram_tensor("s_row2", [4, QB + 128], f32,
                             kind="ExternalInput")
    # cpack cols: selK [128,32] | tri3 [64,192] | uvc [32,256] | cBsB [64,32]
    cp_dram = nc.dram_tensor("cpack", [128, 512], f32, kind="ExternalInput")
    out_dram = nc.dram_tensor("out", [1, 1], f32, kind="ExternalOutput")

    groups = [list(range(NCORES))]

    with tile.TileContext(nc) as tc:
        with tc.tile_pool(name="sbuf", bufs=1) as pool, \
             tc.tile_pool(name="psum", bufs=1, space="PSUM") as psum, \
             tc.tile_pool(name="dram", bufs=1, space="DRAM") as dram:
            # ---------- on-device iota constants (no DMA needed) ----------
            io64 = pool.tile([128, W64], f32)
            nc.gpsimd.iota(io64[:].rearrange("p (b c) -> p b c", c=C2Y),
                           pattern=[[0, NB], [1, C2Y]], base=0,
                           channel_multiplier=0,
                           allow_small_or_imprecise_dtypes=True)
            io32 = pool.tile([128, W32], f32)
            nc.gpsimd.iota(io32[:].rearrange("p (b c) -> p b c", c=C2S),
                           pattern=[[0, NB], [1, C2S]], base=0,
                           channel_multiplier=0,
                           allow_small_or_imprecise_dtypes=True)
            # block-selector: rep64[b, b'*64+c] = [b == b']
            ior64 = pool.tile([NB, W64], f32)
            nc.gpsimd.iota(ior64[:].rearrange("p (b c) -> p b c", c=C2Y),
                           pattern=[[1, NB], [0, C2Y]], base=0,
                           channel_multiplier=-1,
                           allow_small_or_imprecise_dtypes=True)
            ior32 = pool.tile([NB, W32], f32)
            nc.gpsimd.iota(ior32[:].rearrange("p (b c) -> p b c", c=C2S),
                           pattern=[[1, NB], [0, C2S]], base=0,
                           channel_multiplier=-1,
                           allow_small_or_imprecise_dtypes=True)
            rep64 = pool.tile([NB, W64], bf16)
            nc.vector.tensor_scalar(rep64[:], ior64[:], 0.0, None,
                                    ALU.is_equal)
            rep32 = pool.tile([NB, W32], bf16)
            nc.vector.tensor_scalar(rep32[:], ior32[:], 0.0, None,
                                    ALU.is_equal)

            # ---------- input loads (few, fat descriptors) ----------
            qT = pool.tile([NB, 5 * 128], bf16)
            nc.sync.dma_start(qT[:], qt_dram[:])
            s_row = pool.tile([4, QB + 128], f32)
            nc.scalar.dma_start(s_row[:], sr_dram[:])
            cpack = pool.tile([128, 512], f32)
            qeng = [nc.sync, nc.scalar, nc.gpsimd]
            for i in range(8):
                sl = slice(i * 16, (i + 1) * 16)
                qeng[i % 3].dma_start(cpack[sl, :], cp_dram[sl, :])
            selK = cpack[:, 0:32]
            trS = cpack[0:64, 32:96]
            trH = cpack[0:64, 96:160]
            id64 = cpack[0:64, 160:224]
            uvc = cpack[0:K, 224:480]
            cBsB = cpack[0:2 * K, 480:512]

            ones1 = pool.tile([1, 1], f32)
            nc.vector.memset(ones1[:], 1.0)
            lnb1 = pool.tile([1, 1], f32)
            nc.vector.memset(lnb1[:], 1.0)

            # ---------- trig features: s_w = wsel @ s_row, then series ----
            # halves of 320 to keep each PSUM tile within one bank
            sparts = pool.tile([128, 2], f32)
            nc.vector.memset(sparts[:], 0.0)
            cs_ps = psum.tile([K, 2], f32, tag="pcs", bufs=1)
            cos_ins = None
            NH = QB // HQ
            for h in range(NH):
                hs = slice(h * HQ, (h + 1) * HQ)
                pwt = psum.tile([128, 512], f32, tag="pA", bufs=2)
                pw = pwt[:, 0:HQ]
                nc.tensor.matmul(pw, lhsT=s_row[0:4, QB:QB + 128],
                                 rhs=s_row[0:4, hs], start=True, stop=True)
                rnd = pool.tile([128, HQ], f32, tag="rnd", bufs=2)
                nc.scalar.activation(rnd[:], pw, AF.Copy, bias=_MAGIC,
                                     scale=_INV2PI)
                kint = pool.tile([128, HQ], f32, tag="kint", bufs=2)
                nc.vector.tensor_scalar(kint[:], rnd[:], _MAGIC, None,
                                        ALU.subtract)
                sa = pool.tile([128, HQ], f32, tag="sa", bufs=2)
                nc.vector.cody_waite_cascade(sa[:], pw, kint[:],
                                             _CW1, _CW2, _CW3)
                clamp = float(np.float32(_PI))
                nc.vector.tensor_scalar(sa[:], sa[:], clamp, -clamp,
                                        ALU.min, ALU.max)
                ca = pool.tile([128, HQ], f32, tag="ca", bufs=2)
                nc.vector.add_range_wrap(ca[:], sa[:], _PI / 2, _PI, 2 * _PI)
                nc.vector.tensor_scalar(ca[:], ca[:], clamp, -clamp,
                                        ALU.min, ALU.max)
                sin_t = pool.tile([128, HQ], f32, tag="sint", bufs=2)
                nc.scalar.activation(sin_t[:], sa[:], AF.Sin,
                                     accum_out=sparts[:, 0:1])
                cos_t = pool.tile([128, HQ], f32, tag="cost", bufs=2)
                cos_ins = nc.scalar.activation(cos_t[:], ca[:], AF.Sin,
                                               accum_out=sparts[:, 1:2])
                nc.tensor.matmul(cs_ps[:], lhsT=selK, rhs=sparts[:],
                                 start=(h == 0), stop=(h == NH - 1),
                                 skip_group_check=True)
            # switch the ACT table to Ln now; post-AR Ln finds it loaded
            lnwarm = pool.tile([1, 1], f32)
            warm_ins = nc.scalar.activation(lnwarm[:], ones1[:], AF.Ln,
                                            bias=lnb1[:])
            add_dep_helper(warm_ins.ins, cos_ins.ins, False,
                           "Ln table load after the Sin stream")
            cs_sb = pool.tile([K, 2], f32)
            nc.scalar.copy(cs_sb[:], cs_ps[:])

            # ---------- expand ids/weights via PE, build one-hots ----------
            h1y = pool.tile([128, W64], bf16)
            rhs_y = pool.tile([128, 2 * W64], bf16)
            h1s = pool.tile([128, W64], bf16)
            h2se = pool.tile([128, W32], bf16)
            h2sw = pool.tile([128, W32], bf16)

            def expand64(qcol, outs):
                # outs: list of (out_ap_slice_fn, iota_in) consumers per chunk
                for c0 in range(0, W64, 512):
                    c1 = min(c0 + 512, W64)
                    pe = psum.tile([128, 512], f32, tag="pA", bufs=2)
                    nc.tensor.matmul(pe[:, 0:c1 - c0],
                                     lhsT=qT[:, qcol * 128:(qcol + 1) * 128],
                                     rhs=rep64[:, c0:c1],
                                     start=True, stop=True,
                                     skip_group_check=True)
                    yield c0, c1, pe

            # h1y: [q1y == iota]
            for c0, c1, pe in expand64(0, None):
                nc.vector.tensor_tensor(h1y[:, c0:c1], pe[:, 0:c1 - c0],
                                        io64[:, c0:c1], ALU.is_equal)
            # h2y: [q2y == iota] -> rhs_y first half
            for c0, c1, pe in expand64(1, None):
                nc.vector.tensor_tensor(rhs_y[:, c0:c1], pe[:, 0:c1 - c0],
                                        io64[:, c0:c1], ALU.is_equal)
            # y expanded (64-wide) -> weighted second half of rhs_y
            for c0, c1, pe in expand64(4, None):
                nc.vector.tensor_tensor(rhs_y[:, W64 + c0:W64 + c1],
                                        rhs_y[:, c0:c1], pe[:, 0:c1 - c0],
                                        ALU.mult)
            # h1s: [qs1 == iota]
            for c0, c1, pe in expand64(2, None):
                nc.vector.tensor_tensor(h1s[:, c0:c1], pe[:, 0:c1 - c0],
                                        io64[:, c0:c1], ALU.is_equal)
            # s-side level 2: expand qs2 and y at 32-wide, eq + weight
            for c0 in range(0, W32, 320):
                c1 = c0 + 320
                pe = psum.tile([128, 512], f32, tag="pA", bufs=2)
                nc.tensor.matmul(pe[:, 0:320], lhsT=qT[:, 3 * 128:4 * 128],
                                 rhs=rep32[:, c0:c1], start=True, stop=True,
                                 skip_group_check=True)
                nc.vector.tensor_tensor(h2se[:, c0:c1], pe[:, 0:320],
                                        io32[:, c0:c1], ALU.is_equal)
                pe2 = psum.tile([128, 512], f32, tag="pA", bufs=2)
                nc.tensor.matmul(pe2[:, 0:320], lhsT=qT[:, 4 * 128:5 * 128],
                                 rhs=rep32[:, c0:c1], start=True, stop=True,
                                 skip_group_check=True)
                nc.vector.tensor_tensor(h2sw[:, c0:c1], h2se[:, c0:c1],
                                        pe2[:, 0:320], ALU.mult)

            # ---------- weighted histograms (2 interleaved groups) --------
            psY = psum.tile([64, 2 * C2Y], f32, tag="py", bufs=1)
            psSt = psum.tile([64, 96], f32, tag="pC", bufs=2)
            psS = psSt[:, 0:C2S]
            rhs_yv = rhs_y[:].rearrange("p (h x) -> p h x", h=2)
            for b in range(NB):
                nc.tensor.matmul(
                    psY[:], lhsT=h1y[:, b * C2Y:(b + 1) * C2Y],
                    rhs=rhs_yv[:, :, b * C2Y:(b + 1) * C2Y],
                    start=(b == 0), stop=(b == NB - 1),
                    skip_group_check=True)
                nc.tensor.matmul(
                    psS, lhsT=h1s[:, b * C2Y:(b + 1) * C2Y],
                    rhs=h2sw[:, b * C2S:(b + 1) * C2S],
                    start=(b == 0), stop=(b == NB - 1),
                    skip_group_check=True)

            # ---------- local suffix table T ----------
            hist_sb = pool.tile([64, C2Y], f32)
            nc.scalar.copy(hist_sb[:], psY[:, 0:C2Y])
            ysum_sb = pool.tile([64, C2Y], f32)
            nc.scalar.copy(ysum_sb[:], psY[:, C2Y:2 * C2Y])
            ys_sb = pool.tile([64, C2S], f32)
            nc.scalar.copy(ys_sb[:], psS)
            rowsum = pool.tile([64, 1], f32)
            nc.vector.tensor_reduce(rowsum[:], psY[:, 0:C2Y], axis=X,
                                    op=ALU.add)
            htpt = psum.tile([64, 96], f32, tag="pC", bufs=2)
            htp = htpt[:, 0:64]
            nc.tensor.transpose(htp, hist_sb[:], id64)
            hts = pool.tile([64, 64], f32)
            nc.scalar.copy(hts[:], htp)
            srfx = psum.tile([64, 96], f32, tag="pC", bufs=2)
            nc.tensor.matmul(srfx[:, 0:64], lhsT=hts[:], rhs=trH,
                             start=True, stop=True, skip_group_check=True)
            nc.tensor.matmul(srfx[:, 64:65], lhsT=trS, rhs=rowsum[:],
                             start=True, stop=True, skip_group_check=True)
            t_loc = pool.tile([64, C2Y], f32)
            nc.vector.tensor_scalar(t_loc[:], srfx[:, 0:64],
                                    srfx[:, 64:65], None, ALU.add)

            # ---------- fused AllReduce: [T | Ysum | Ys | C/S] ----------
            cc_in = dram.tile([64, 162], f32)
            cc_out = dram.tile([64, 162], f32)
            z32 = pool.tile([K, 2], f32)
            nc.vector.memset(z32[:], 0.0)
            nc.scalar.dma_start(cc_in[K:2 * K, 160:162], z32[:])
            nc.sync.dma_start(cc_in[:, 0:64], t_loc[:])
            nc.sync.dma_start(cc_in[:, 64:128], ysum_sb[:])
            nc.scalar.dma_start(cc_in[:, 128:160], ys_sb[:])
            nc.scalar.dma_start(cc_in[0:K, 160:162], cs_sb[:])
            nc.gpsimd.collective_compute(
                "AllReduce", ALU.add, replica_groups=groups,
                ins=[cc_in[:, :].opt()], outs=[cc_out[:, :].opt()])
            t_glob = pool.tile([64, C2Y], f32)
            nc.sync.dma_start(t_glob[:], cc_out[:, 0:64])
            ysg = pool.tile([64, C2Y], f32)
            nc.sync.dma_start(ysg[:], cc_out[:, 64:128])
            yss = pool.tile([64, C2S], f32)
            nc.scalar.dma_start(yss[:], cc_out[:, 128:160])
            csg = pool.tile([K, 2], f32)
            nc.scalar.dma_start(csg[:], cc_out[0:K, 160:162])

            # ---------- dcg: series at score-bucket centers ----------
            # csg col0 = S_k, col1 = C_k; pads contribute cos(0)=1 each
            nc.vector.tensor_scalar(csg[:, 1:2], csg[:, 1:2],
                                    float(TRIG_PAD), None, ALU.subtract)
            # luv rows 0:K = -U, K:2K = -V  (negated; Ln uses scale=-1)
            luv = pool.tile([2 * K, 64], f32)
            u1 = pool.tile([K, 64], f32)
            nc.vector.tensor_scalar(u1[:], uvc[:, 0:64], csg[:, 1:2], None,
                                    ALU.mult)
            nc.vector.scalar_tensor_tensor(luv[0:K, :], uvc[:, 64:128],
                                           csg[:, 0:1], u1[:],
                                           ALU.mult, ALU.subtract)
            v1 = pool.tile([K, 64], f32)
            nc.vector.tensor_scalar(v1[:], uvc[:, 192:256], csg[:, 1:2],
                                    None, ALU.mult)
            nc.vector.scalar_tensor_tensor(luv[K:2 * K, :], uvc[:, 128:192],
                                           csg[:, 0:1], v1[:],
                                           ALU.mult, ALU.add)
            rank_pst = psum.tile([64, 96], f32, tag="pC", bufs=2)
            rank_ps = rank_pst[:, 0:C2S]
            nc.tensor.matmul(rank_ps, lhsT=luv[:], rhs=cBsB,
                             start=True, stop=True)
            dbias = pool.tile([64, 1], f32)
            nc.vector.memset(dbias[:], float(N / 2 + 2.0))
            ibias = pool.tile([64, 1], f32)
            nc.vector.memset(ibias[:], 1.5)
            lnds = pool.tile([64, C2S], f32)
            nc.scalar.activation(lnds[:], rank_ps, AF.Ln,
                                 bias=dbias[:], scale=-1.0)
            rds = pool.tile([64, C2S], f32)
            nc.vector.reciprocal(rds[:], lnds[:])
            parts = pool.tile([64, 3], f32)
            scrD = pool.tile([64, C2S], f32)
            nc.vector.scalar_tensor_tensor(scrD[:], yss[:], LN2, rds[:],
                                           ALU.mult, ALU.mult,
                                           accum_out=parts[:, 0:1])
            # ---------- idcg: per-bucket mid-rank discount ----------
            lnis = pool.tile([64, C2Y], f32)
            nc.scalar.activation(lnis[:], t_glob[:], AF.Ln, bias=ibias[:])
            ris = pool.tile([64, C2Y], f32)
            nc.vector.reciprocal(ris[:], lnis[:])
            scrI = pool.tile([64, C2Y], f32)
            nc.vector.scalar_tensor_tensor(scrI[:], ysg[:], LN2, ris[:],
                                           ALU.mult, ALU.mult,
                                           accum_out=parts[:, 1:2])
            nc.vector.tensor_reduce(parts[:, 2:3], ysg[:], axis=X,
                                    op=ALU.add)

            # ---------- fold partitions, final scalar loss ----------
            ones64 = pool.tile([64, 1], f32)
            nc.vector.memset(ones64[:], 1.0)
            ps2t = psum.tile([64, 96], f32, tag="pC", bufs=2)
            ps2 = ps2t[0:1, 0:3]
            nc.tensor.matmul(ps2, lhsT=ones64[:], rhs=parts[:],
                             start=True, stop=True)
            d1 = pool.tile([1, 1], f32)
            nc.vector.tensor_scalar(d1[:], ps2t[0:1, 1:2], 1e-8, None,
                                    ALU.add)
            rec = pool.tile([1, 1], f32)
            nc.vector.reciprocal(rec[:], d1[:])
            negl = pool.tile([1, 1], f32)
            nc.vector.scalar_tensor_tensor(negl[:], ps2t[0:1, 0:1], rec[:],
                                           ones1[:], ALU.mult, ALU.subtract)
            negm = pool.tile([1, 1], f32)
            nc.vector.tensor_scalar(negm[:], ps2t[0:1, 2:3], 1.0, -1.0,
                                    ALU.is_ge, ALU.mult)
            fin = pool.tile([1, 1], f32)
            nc.vector.tensor_tensor(fin[:], negl[:], negm[:], ALU.mult)
            nc.sync.dma_start(out_dram[:], fin[:])

    nc.compile()
    return nc


def _get_nc():
    if "nc" not in _CACHE:
        _CACHE["nc"] = _build()
    return _CACHE["nc"]


def _consts():
    p = np.arange(128)
    selK = (p[:, None] // 4 == np.arange(K)[None, :]).astype(np.float32)
    a = np.arange(64)
    trS = (a[:, None] > a[None, :]).astype(np.float32)
    trH = ((a[:, None] > a[None, :]).astype(np.float32)
           + 0.5 * (a[:, None] == a[None, :]).astype(np.float32))
    id64 = np.eye(64, dtype=np.float32)
    om = _OMEGA.astype(np.float64)[:, None]
    aang = om * (LO + np.arange(64, dtype=np.float64)[None, :] * C2S * DELTA)
    bang = om * ((np.arange(C2S, dtype=np.float64)[None, :] + 0.5) * DELTA)
    bk = _B.astype(np.float64)[:, None]
    sAb = (bk * np.sin(aang)).astype(np.float32)
    cAb = (bk * np.cos(aang)).astype(np.float32)
    uvc = np.concatenate([sAb, cAb, -sAb, -cAb], axis=1)
    cBsB = np.concatenate([np.cos(bang), np.sin(bang)],
                          axis=0).astype(np.float32)
    cpack = np.zeros((128, 512), np.float32)
    cpack[:, 0:32] = selK
    cpack[0:64, 32:96] = trS
    cpack[0:64, 96:160] = trH
    cpack[0:64, 160:224] = id64
    cpack[0:K, 224:480] = uvc
    cpack[0:2 * K, 480:512] = cBsB
    return np.ascontiguousarray(cpack)


def _in_maps(logits, targets):
    s = np.asarray(logits, dtype=np.float32).reshape(-1)
    y = np.asarray(targets, dtype=np.float32).reshape(-1)
    npad = NCORES * PB
    s_pad = np.zeros((npad,), np.float32)
    s_pad[:N] = s
    y_pad = np.zeros((npad,), np.float32)
    y_pad[:N] = y
    q = np.clip(np.floor(y.astype(np.float64) * QSY).astype(np.int64),
                0, QSY - 1)
    qy1_pad = np.full((npad,), -1.0, np.float32)
    qy1_pad[:N] = (q // C2Y).astype(np.float32)
    qy2_pad = np.full((npad,), -1.0, np.float32)
    qy2_pad[:N] = (q % C2Y).astype(np.float32)
    qs = np.clip(np.floor((s.astype(np.float64) - LO) / DELTA).astype(
        np.int64), 0, MBS - 1)
    qs1_pad = np.full((npad,), -1.0, np.float32)
    qs1_pad[:N] = (qs // C2S).astype(np.float32)
    qs2_pad = np.full((npad,), -1.0, np.float32)
    qs2_pad[:N] = (qs % C2S).astype(np.float32)
    cpack = _consts()
    # omega selector: wsel[bh, p] = OMEGA[p//4] * (bh == p%4)
    pp = np.arange(128)
    wsel = (_OMEGA[pp // 4][None, :]
            * (np.arange(4)[:, None] == pp[None, :] % 4)).astype(np.float32)
    maps = []
    for d in range(NCORES):
        sl = slice(d * PB, (d + 1) * PB)
        sv = s_pad[sl]
        s_row2 = np.concatenate([sv.reshape(4, QB), wsel],
                                axis=1).astype(np.float32)
        # per-block lhsT layout: qT[b, p] = value of item b*128+p
        qT = np.concatenate([
            qy1_pad[sl].reshape(NB, 128), qy2_pad[sl].reshape(NB, 128),
            qs1_pad[sl].reshape(NB, 128), qs2_pad[sl].reshape(NB, 128),
            y_pad[sl].reshape(NB, 128)], axis=1)
        maps.append({
            "qT": np.ascontiguousarray(qT).astype(ml_dtypes.bfloat16),
            "s_row2": np.ascontiguousarray(s_row2),
            "cpack": cpack,
        })
    return maps


def kernel(logits, targets):
    nc = _get_nc()
    res = run_bass_kernel_spmd(nc, _in_maps(logits, targets),
                               core_ids=list(range(NCORES)))
    out = np.asarray(res.results[0]["out"], dtype=np.float32)
    return out.reshape(())


# revision 22
# speedup vs baseline: 1.0528x; 1.0528x over previous
"""ApproxNDCGLoss on 8 TRN2 NeuronCores (Bass/Tile) — one-collective design.

loss = 1 - dcg/(idcg+1e-8):
  approx_rank[j] = 1 + sum_i sigmoid(s[j]-s[i])
  dcg  = sum_j y[j] / log2(approx_rank[j]+1)
  idcg = sum_j y[j] / log2(rank_y[j]+1),  rank_y[j] = 1 + #{i: y[i] > y[j]}

Everything per-item is folded into per-BUCKET sums that are additive across
cores, so a single fused AllReduce replaces the old (T-table AR + per-item
lookup + partials AR) pipeline:

DCG:  sigmoid(x) - 1/2 ~= sum_k b_k sin(w_k x)  (K=32 sine series), so
  rank(t)+1 = n/2 + 2 + sum_k b_k [sin(w_k t) C_k - cos(w_k t) S_k]
  with C/S = global trig sums.  Scores are binned into 1024 buckets
  (32x32 two-level); per-bucket y-sums Ys are AllReduced, and post-AR the
  series is evaluated at all bucket centers with ONE 64-contraction matmul
  via the angle-addition split  theta = A(c1) + B(c2):
      ser[c1,c2] = sum_k U_k(c1) cosB_k(c2) + V_k(c1) sinB_k(c2)
  where U,V are [32,32] tiles built from (C,S) and host trig constants.
  dcg = sum_b Ys_b * ln2/ln(ser_b + n/2 + 2).  Bucketing error ~1e-5.

IDCG: y in [0,1) binned into 2048 buckets (64x32).  Joint histogram via
  one-hot matmuls; suffix-count table T (strict suffix + hist/2) is linear
  in hist so each core builds its local T pre-collective.  Per-bucket
  y-sums Ysum give  idcg = sum_b Ysum_b * ln2/ln(T_b + 1.5).

The collectives share DMA bandwidth with input loads across all 8 cores,
so the mesh start is gated by the slowest core's (launch stagger + local
compute), and input bytes burn shared DMA time.  Only ~220KB/core is
shipped: per-block ids/weights qT [21,640], raw quad scores [4,768], and
packed constants [128,352].  On device, PE outer-product matmuls against
an iota-built block-selector expand ids to (q - class) differences in one
shot (an extra ones-row in lhsT folds the -iota right into the matmul), a
single IS_EQ-with-0 then yields each bf16 one-hot; w_k*s comes from a
4-contraction matmul against an omega-selector.  The y-expansion is built
once at 32 wide and shared by both weighted histograms.  No partition
broadcasts, no per-item post-AR work, fat DMA descriptors only.
"""

import numpy as np
import ml_dtypes

import concourse.bacc as bacc
import concourse.bass as bass
import concourse.mybir as mybir
import concourse.tile as tile
from concourse.bass_utils import run_bass_kernel_spmd
from concourse.tile_rust import add_dep_helper

N = 20000
NCORES = 8
PB = 2560                   # items per core (padded; 8*2560 = 20480)
NB = PB // 128              # 20 column blocks of 128 items
QB = PB // 4                # 640 free elems in the quad trig layout
HQ = QB // 2                # trig processed in two 320-wide PSUM halves
K = 32                      # Fourier terms
L = 24.2                    # period of the sine series
TRIG_PAD = NCORES * PB - N  # 480 zero-score pads -> C_k -= 480
# y buckets: 2048 = 64 (partitions) x 32 (free)
QSY = 2048
C1Y = 64
C2Y = 32
W64 = NB * C1Y              # 1280 (level-1 one-hot width)
W32 = NB * C2Y              # 640  (level-2 / 32-class one-hot width)
# score buckets: 1024 = 32 x 32
MBS = 1024
C1S = 32
C2S = 32
LO, HI = -5.5, 5.5
DELTA = (HI - LO) / MBS
LN2 = float(np.log(2.0))

_B = np.array([
    0.575840175151825, -0.0012469458160921931, 0.08171718567609787,
    0.019092485308647156, -0.007231124211102724, 0.02490580640733242,
    -0.017197489738464355, 0.014312449842691422, -0.007428332697600126,
    0.003442077897489071, -0.0007101596565917134, 3.444465983193368e-05,
    -0.00029458850622177124, 0.0009411321370862424, -0.0013493510195985436,
    0.0013473577564582229, -0.0009938474977388978, 0.0005221660248935223,
    -0.00015226299001369625, 2.9422192255879054e-06, -5.903289275011048e-05,
    0.00021578818268608302, -0.0003499265294522047, 0.0003830934874713421,
    -0.00030826698639430106, 0.0001763014297466725, -5.747509567299858e-05,
    2.007998773478903e-06, -1.8746375644695945e-05, 7.875602022977546e-05,
    -0.00013714544184040278, 0.00015883310697972775], dtype=np.float32)
_OMEGA = (2.0 * np.pi * np.arange(1, K + 1) / L).astype(np.float32)

# range reduction: m = x - round(x/2pi)*2pi via magic-number round and a
# 3-term Cody-Waite cascade.
_MAGIC = float(np.float32(1.5 * 2.0 ** 23))
_INV2PI = float(np.float32(1.0 / (2.0 * np.pi)))
_CW1 = 6.28125
_CW2 = float(np.float32(2.0 * np.pi - 6.28125))
_CW3 = float(np.float32(2.0 * np.pi - 6.28125
                        - np.float64(np.float32(2.0 * np.pi - 6.28125))))
_PI = float(np.pi)

_CACHE = {}


def _build():
    f32 = mybir.dt.float32
    bf16 = mybir.dt.bfloat16
    AF = mybir.ActivationFunctionType
    ALU = mybir.AluOpType
    X = mybir.AxisListType.X

    nc = bacc.Bacc("TRN2", target_bir_lowering=False, debug=False,
                   num_devices=NCORES)
    # qT col-groups of 128: q1y | q2y | qs1 | qs2 | y.  Row 20 = ones (the
    # -iota row of the rep operand turns the expansion into q - class).
    qt_dram = nc.dram_tensor("qT", [NB + 1, 5 * 128], bf16,
                             kind="ExternalInput")
    # negio: cols 0:W64 = -(iota64 tiled), W64:W64+W32 = -(iota32 tiled)
    ng_dram = nc.dram_tensor("negio", [1, W64 + W32], bf16,
                             kind="ExternalInput")
    # s_row2: quad scores [4, 640] | omega-selector wsel [4, 128]
    sr_dram = nc.dram_tensor("s_row2", [4, QB + 128], f32,
                             kind="ExternalInput")
    # cpack cols: selK[128,32] | trS[64,64] | trH[32,32] | id64[64,64] |
    #             uvc[32,128] | cBsB[64,32]
    cp_dram = nc.dram_tensor("cpack", [128, 352], f32, kind="ExternalInput")
    out_dram = nc.dram_tensor("out", [1, 1], f32, kind="ExternalOutput")

    groups = [list(range(NCORES))]

    with tile.TileContext(nc) as tc:
        with tc.tile_pool(name="sbuf", bufs=1) as pool, \
             tc.tile_pool(name="psum", bufs=1, space="PSUM") as psum, \
             tc.tile_pool(name="dram", bufs=1, space="DRAM") as dram:
            # ---------- block-selector iotas (gpsimd, no deps) ----------
            ior64 = pool.tile([NB, W64], f32)
            nc.gpsimd.iota(ior64[:].rearrange("p (b c) -> p b c", c=C1Y),
                           pattern=[[1, NB], [0, C1Y]], base=0,
                           channel_multiplier=-1,
                           allow_small_or_imprecise_dtypes=True)
            ior32 = pool.tile([NB, W32], f32)
            nc.gpsimd.iota(ior32[:].rearrange("p (b c) -> p b c", c=C2Y),
                           pattern=[[1, NB], [0, C2Y]], base=0,
                           channel_multiplier=-1,
                           allow_small_or_imprecise_dtypes=True)

            # ---------- input loads (sync/scalar queues only) ----------
            qT = pool.tile([NB + 1, 5 * 128], bf16)
            nc.sync.dma_start(qT[:], qt_dram[:])
            negio = pool.tile([1, W64 + W32], bf16)
            nc.sync.dma_start(negio[:], ng_dram[:])
            s_row = pool.tile([4, QB + 128], f32)
            nc.scalar.dma_start(s_row[:], sr_dram[:])
            cpack = pool.tile([128, 352], f32)
            qeng = [nc.sync, nc.scalar]
            for i in range(4):
                sl = slice(i * 32, (i + 1) * 32)
                qeng[i % 2].dma_start(cpack[sl, :], cp_dram[sl, :])
            selK = cpack[:, 0:32]
            trS = cpack[0:64, 32:96]
            trH = cpack[0:C2Y, 96:128]
            id64 = cpack[0:64, 128:192]
            uvc = cpack[0:K, 192:320]
            cBsB = cpack[0:2 * K, 320:352]

            # rep operands: rows 0:NB one-hot block selector, row NB = -iota
            rep64 = pool.tile([NB + 1, W64], bf16)
            nc.vector.tensor_scalar(rep64[0:NB, :], ior64[:], 0.0, None,
                                    ALU.is_equal)
            nc.sync.dma_start(rep64[NB:NB + 1, :], negio[:, 0:W64])
            rep32 = pool.tile([NB + 1, W32], bf16)
            nc.vector.tensor_scalar(rep32[0:NB, :], ior32[:], 0.0, None,
                                    ALU.is_equal)
            nc.scalar.dma_start(rep32[NB:NB + 1, :], negio[:, W64:])

            ones1 = pool.tile([1, 1], f32)
            nc.vector.memset(ones1[:], 1.0)
            lnb1 = pool.tile([1, 1], f32)
            nc.vector.memset(lnb1[:], 1.0)

            # ---------- trig features: s_w = wsel @ s_row, then series ----
            sparts = pool.tile([128, 2], f32)
            nc.vector.memset(sparts[:], 0.0)
            cos_ins = None
            cs_ps = psum.tile([K, 2], f32, tag="pcs", bufs=1)
            for h in range(2):
                hs = slice(h * HQ, (h + 1) * HQ)
                pwt = psum.tile([128, 512], f32, tag="pA", bufs=2)
                pw = pwt[:, 0:HQ]
                nc.tensor.matmul(pw, lhsT=s_row[0:4, QB:QB + 128],
                                 rhs=s_row[0:4, hs], start=True, stop=True)
                rnd = pool.tile([128, HQ], f32, tag="rnd", bufs=2)
                nc.scalar.activation(rnd[:], pw, AF.Copy, bias=_MAGIC,
                                     scale=_INV2PI)
                kint = pool.tile([128, HQ], f32, tag="kint", bufs=2)
                nc.vector.tensor_scalar(kint[:], rnd[:], _MAGIC, None,
                                        ALU.subtract)
                sa = pool.tile([128, HQ], f32, tag="sa", bufs=2)
                nc.vector.cody_waite_cascade(sa[:], pw, kint[:],
                                             _CW1, _CW2, _CW3)
                clamp = float(np.float32(_PI))
                nc.vector.tensor_scalar(sa[:], sa[:], clamp, -clamp,
                                        ALU.min, ALU.max)
                ca = pool.tile([128, HQ], f32, tag="ca", bufs=2)
                nc.vector.add_range_wrap(ca[:], sa[:], _PI / 2, _PI, 2 * _PI)
                nc.vector.tensor_scalar(ca[:], ca[:], clamp, -clamp,
                                        ALU.min, ALU.max)
                sin_t = pool.tile([128, HQ], f32, tag="sint", bufs=2)
                nc.scalar.activation(sin_t[:], sa[:], AF.Sin,
                                     accum_out=sparts[:, 0:1])
                cos_t = pool.tile([128, HQ], f32, tag="cost", bufs=2)
                cos_ins = nc.scalar.activation(cos_t[:], ca[:], AF.Sin,
                                               accum_out=sparts[:, 1:2])
                nc.tensor.matmul(cs_ps[:], lhsT=selK, rhs=sparts[:],
                                 start=(h == 0), stop=(h == 1),
                                 skip_group_check=True)
            # switch the ACT table to Ln now; post-AR Ln finds it loaded
            lnwarm = pool.tile([1, 1], f32)
            warm_ins = nc.scalar.activation(lnwarm[:], ones1[:], AF.Ln,
                                            bias=lnb1[:])
            add_dep_helper(warm_ins.ins, cos_ins.ins, False,
                           "Ln table load after the Sin stream")
            cs_sb = pool.tile([K, 2], f32)
            nc.scalar.copy(cs_sb[:], cs_ps[:])

            # ---------- expansions (q - class into PSUM) + one-hots -------
            h1y = pool.tile([128, W64], bf16)
            rhs_y = pool.tile([128, 2 * W32], bf16)
            yx = pool.tile([128, W32], bf16)
            h1s = pool.tile([128, W32], bf16)
            h2se = pool.tile([128, W32], bf16)
            h2sw = pool.tile([128, W32], bf16)

            # q1y: 64-class, 3 chunks
            for c0, c1 in ((0, 512), (512, 1024), (1024, W64)):
                pe = psum.tile([128, 512], f32, tag="pA", bufs=2)
                nc.tensor.matmul(pe[:, 0:c1 - c0], lhsT=qT[:, 0:128],
                                 rhs=rep64[:, c0:c1], start=True, stop=True,
                                 skip_group_check=True)
                nc.vector.tensor_scalar(h1y[:, c0:c1], pe[:, 0:c1 - c0],
                                        0.0, None, ALU.is_equal)
            # q2y + shared y expansion + weighted half, 2 chunks of 320
            for c0 in (0, 320):
                c1 = c0 + 320
                pe = psum.tile([128, 512], f32, tag="pA", bufs=2)
                nc.tensor.matmul(pe[:, 0:320], lhsT=qT[:, 128:256],
                                 rhs=rep32[:, c0:c1], start=True, stop=True,
                                 skip_group_check=True)
                nc.vector.tensor_scalar(rhs_y[:, c0:c1], pe[:, 0:320],
                                        0.0, None, ALU.is_equal)
                pe2 = psum.tile([128, 512], f32, tag="pA", bufs=2)
                nc.tensor.matmul(pe2[:, 0:320], lhsT=qT[0:NB, 512:640],
                                 rhs=rep32[0:NB, c0:c1], start=True,
                                 stop=True, skip_group_check=True)
                nc.vector.tensor_scalar(yx[:, c0:c1], pe2[:, 0:320], 1.0,
                                        None, ALU.mult)
                nc.vector.tensor_tensor(rhs_y[:, W32 + c0:W32 + c1],
                                        rhs_y[:, c0:c1], yx[:, c0:c1],
                                        ALU.mult)
            # qs1 / qs2, 2 chunks each; weighted s-side on gpsimd
            for c0 in (0, 320):
                c1 = c0 + 320
                pe = psum.tile([128, 512], f32, tag="pA", bufs=2)
                nc.tensor.matmul(pe[:, 0:320], lhsT=qT[:, 256:384],
                                 rhs=rep32[:, c0:c1], start=True, stop=True,
                                 skip_group_check=True)
                nc.vector.tensor_scalar(h1s[:, c0:c1], pe[:, 0:320],
                                        0.0, None, ALU.is_equal)
                pe2 = psum.tile([128, 512], f32, tag="pA", bufs=2)
                nc.tensor.matmul(pe2[:, 0:320], lhsT=qT[:, 384:512],
                                 rhs=rep32[:, c0:c1], start=True, stop=True,
                                 skip_group_check=True)
                nc.vector.tensor_scalar(h2se[:, c0:c1], pe2[:, 0:320],
                                        0.0, None, ALU.is_equal)
                nc.gpsimd.tensor_tensor(h2sw[:, c0:c1], h2se[:, c0:c1],
                                        yx[:, c0:c1], ALU.mult)

            # ---------- weighted histograms (2 interleaved groups) --------
            psY = psum.tile([C1Y, 2 * C2Y], f32, tag="py", bufs=1)
            psSt = psum.tile([64, 96], f32, tag="pC", bufs=2)
            psS = psSt[0:C1S, 0:C2S]
            rhs_yv = rhs_y[:].rearrange("p (h x) -> p h x", h=2)
            for b in range(NB):
                nc.tensor.matmul(
                    psY[:], lhsT=h1y[:, b * C1Y:(b + 1) * C1Y],
                    rhs=rhs_yv[:, :, b * C2Y:(b + 1) * C2Y],
                    start=(b == 0), stop=(b == NB - 1),
                    skip_group_check=True)
                nc.tensor.matmul(
                    psS, lhsT=h1s[:, b * C1S:(b + 1) * C1S],
                    rhs=h2sw[:, b * C2S:(b + 1) * C2S],
                    start=(b == 0), stop=(b == NB - 1),
                    skip_group_check=True)

            # ---------- local suffix table T ----------
            hist_sb = pool.tile([C1Y, C2Y], f32)
            nc.scalar.copy(hist_sb[:], psY[:, 0:C2Y])
            ysum_sb = pool.tile([C1Y, C2Y], f32)
            nc.scalar.copy(ysum_sb[:], psY[:, C2Y:2 * C2Y])
            ys_sb = pool.tile([C1S, C2S], f32)
            nc.scalar.copy(ys_sb[:], psS)
            rowsum = pool.tile([C1Y, 1], f32)
            nc.vector.tensor_reduce(rowsum[:], psY[:, 0:C2Y], axis=X,
                                    op=ALU.add)
            htpt = psum.tile([64, 96], f32, tag="pC", bufs=2)
            htp = htpt[0:C2Y, 0:C1Y]
            nc.tensor.transpose(htp, hist_sb[:], id64)
            hts = pool.tile([C2Y, C1Y], f32)
            nc.scalar.copy(hts[:], htp)
            srfx = psum.tile([64, 96], f32, tag="pC", bufs=2)
            nc.tensor.matmul(srfx[:, 0:C2Y], lhsT=hts[:], rhs=trH,
                             start=True, stop=True, skip_group_check=True)
            nc.tensor.matmul(srfx[:, 64:65], lhsT=trS, rhs=rowsum[:],
                             start=True, stop=True, skip_group_check=True)
            t_loc = pool.tile([C1Y, C2Y], f32)
            nc.vector.tensor_scalar(t_loc[:], srfx[:, 0:C2Y],
                                    srfx[:, 64:65], None, ALU.add)

            # ---------- fused AllReduce: [T | Ysum | Ys | C/S] ----------
            PWC = C2Y + C2Y + C2S + 2          # 98 payload cols
            cc_in = dram.tile([C1Y, PWC], f32)
            cc_out = dram.tile([C1Y, PWC], f32, addr_space="Shared")
            zfill = pool.tile([C1Y - C1S, C2S + 2], f32)
            nc.vector.memset(zfill[:], 0.0)
            nc.scalar.dma_start(cc_in[C1S:C1Y, 2 * C2Y:PWC], zfill[:])
            nc.sync.dma_start(cc_in[:, 0:C2Y], t_loc[:])
            nc.sync.dma_start(cc_in[:, C2Y:2 * C2Y], ysum_sb[:])
            nc.scalar.dma_start(cc_in[0:C1S, 2 * C2Y:2 * C2Y + C2S],
                                ys_sb[:])
            nc.scalar.dma_start(cc_in[0:K, 2 * C2Y + C2S:PWC], cs_sb[:])
            nc.gpsimd.collective_compute(
                "AllReduce", ALU.add, replica_groups=groups,
                ins=[cc_in[:, :].opt()], outs=[cc_out[:, :].opt()])
            t_glob = pool.tile([C1Y, C2Y], f32)
            nc.sync.dma_start(t_glob[:], cc_out[:, 0:C2Y])
            ysg = pool.tile([C1Y, C2Y], f32)
            nc.sync.dma_start(ysg[:], cc_out[:, C2Y:2 * C2Y])
            yss = pool.tile([C1S, C2S], f32)
            nc.scalar.dma_start(yss[:], cc_out[0:C1S, 2 * C2Y:2 * C2Y + C2S])
            csg = pool.tile([K, 2], f32)
            nc.scalar.dma_start(csg[:], cc_out[0:K, 2 * C2Y + C2S:PWC])

            # ---------- dcg: series at score-bucket centers ----------
            # csg col0 = S_k, col1 = C_k; pads contribute cos(0)=1 each
            nc.vector.tensor_scalar(csg[:, 1:2], csg[:, 1:2],
                                    float(TRIG_PAD), None, ALU.subtract)
            # luv rows 0:K = -U, K:2K = -V  (negated; Ln uses scale=-1)
            luv = pool.tile([2 * K, C1S], f32)
            u1 = pool.tile([K, C1S], f32)
            nc.vector.tensor_scalar(u1[:], uvc[:, 0:32], csg[:, 1:2], None,
                                    ALU.mult)
            nc.vector.scalar_tensor_tensor(luv[0:K, :], uvc[:, 32:64],
                                           csg[:, 0:1], u1[:],
                                           ALU.mult, ALU.subtract)
            v1 = pool.tile([K, C1S], f32)
            nc.vector.tensor_scalar(v1[:], uvc[:, 96:128], csg[:, 1:2],
                                    None, ALU.mult)
            nc.vector.scalar_tensor_tensor(luv[K:2 * K, :], uvc[:, 64:96],
                                           csg[:, 0:1], v1[:],
                                           ALU.mult, ALU.add)
            rank_pst = psum.tile([64, 96], f32, tag="pC", bufs=2)
            rank_ps = rank_pst[0:C1S, 0:C2S]
            nc.tensor.matmul(rank_ps, lhsT=luv[:], rhs=cBsB,
                             start=True, stop=True)
            dbias = pool.tile([C1S, 1], f32)
            nc.vector.memset(dbias[:], float(N / 2 + 2.0))
            ibias = pool.tile([C1Y, 1], f32)
            nc.vector.memset(ibias[:], 1.5)
            parts = pool.tile([64, 3], f32)
            nc.vector.memset(parts[:], 0.0)
            lnds = pool.tile([C1S, C2S], f32)
            nc.scalar.activation(lnds[:], rank_ps, AF.Ln,
                                 bias=dbias[:], scale=-1.0)
            rds = pool.tile([C1S, C2S], f32)
            nc.vector.reciprocal(rds[:], lnds[:])
            scrD = pool.tile([C1S, C2S], f32)
            nc.vector.scalar_tensor_tensor(scrD[:], yss[:], LN2, rds[:],
                                           ALU.mult, ALU.mult,
                                           accum_out=parts[0:C1S, 0:1])
            # ---------- idcg: per-bucket mid-rank discount ----------
            lnis = pool.tile([C1Y, C2Y], f32)
            nc.scalar.activation(lnis[:], t_glob[:], AF.Ln, bias=ibias[:])
            ris = pool.tile([C1Y, C2Y], f32)
            nc.vector.reciprocal(ris[:], lnis[:])
            scrI = pool.tile([C1Y, C2Y], f32)
            nc.vector.scalar_tensor_tensor(scrI[:], ysg[:], LN2, ris[:],
                                           ALU.mult, ALU.mult,
                                           accum_out=parts[:, 1:2])
            nc.vector.tensor_reduce(parts[:, 2:3], ysg[:], axis=X,
                                    op=ALU.add)

            # ---------- fold partitions, final scalar loss ----------
            ones64 = pool.tile([64, 1], f32)
            nc.vector.memset(ones64[:], 1.0)
            ps2t = psum.tile([64, 96], f32, tag="pC", bufs=2)
            ps2 = ps2t[0:1, 0:3]
            nc.tensor.matmul(ps2, lhsT=ones64[:], rhs=parts[:],
                             start=True, stop=True)
            d1 = pool.tile([1, 1], f32)
            nc.vector.tensor_scalar(d1[:], ps2t[0:1, 1:2], 1e-8, None,
                                    ALU.add)
            rec = pool.tile([1, 1], f32)
            nc.vector.reciprocal(rec[:], d1[:])
            negl = pool.tile([1, 1], f32)
            nc.vector.scalar_tensor_tensor(negl[:], ps2t[0:1, 0:1], rec[:],
                                           ones1[:], ALU.mult, ALU.subtract)
            negm = pool.tile([1, 1], f32)
            nc.vector.tensor_scalar(negm[:], ps2t[0:1, 2:3], 1.0, -1.0,
                                    ALU.is_ge, ALU.mult)
            fin = pool.tile([1, 1], f32)
            nc.vector.tensor_tensor(fin[:], negl[:], negm[:], ALU.mult)
            nc.sync.dma_start(out_dram[:], fin[:])

    nc.compile()
    return nc


def _get_nc():
    if "nc" not in _CACHE:
        _CACHE["nc"] = _build()
    return _CACHE["nc"]


def _consts():
    p = np.arange(128)
    selK = (p[:, None] // 4 == np.arange(K)[None, :]).astype(np.float32)
    a = np.arange(64)
    trS = (a[:, None] > a[None, :]).astype(np.float32)
    a2 = np.arange(C2Y)
    trH = ((a2[:, None] > a2[None, :]).astype(np.float32)
           + 0.5 * (a2[:, None] == a2[None, :]).astype(np.float32))
    id64 = np.eye(64, dtype=np.float32)
    om = _OMEGA.astype(np.float64)[:, None]
    aang = om * (LO + np.arange(C1S, dtype=np.float64)[None, :]
                 * C2S * DELTA)
    bang = om * ((np.arange(C2S, dtype=np.float64)[None, :] + 0.5) * DELTA)
    bk = _B.astype(np.float64)[:, None]
    sAb = (bk * np.sin(aang)).astype(np.float32)
    cAb = (bk * np.cos(aang)).astype(np.float32)
    # device layout: 0:32 sAb, 32:64 cAb, 64:96 -sAb, 96:128 -cAb
    uvc = np.concatenate([sAb, cAb, -sAb, -cAb], axis=1)
    cBsB = np.concatenate([np.cos(bang), np.sin(bang)],
                          axis=0).astype(np.float32)
    cpack = np.zeros((128, 352), np.float32)
    cpack[:, 0:32] = selK
    cpack[0:64, 32:96] = trS
    cpack[0:C2Y, 96:128] = trH
    cpack[0:64, 128:192] = id64
    cpack[0:K, 192:320] = uvc
    cpack[0:2 * K, 320:352] = cBsB
    negio = np.concatenate([
        -np.tile(np.arange(C1Y, dtype=np.float32), NB),
        -np.tile(np.arange(C2Y, dtype=np.float32), NB)]).reshape(1, -1)
    return np.ascontiguousarray(cpack), \
        np.ascontiguousarray(negio).astype(ml_dtypes.bfloat16)


def _in_maps(logits, targets):
    s = np.asarray(logits, dtype=np.float32).reshape(-1)
    y = np.asarray(targets, dtype=np.float32).reshape(-1)
    npad = NCORES * PB
    s_pad = np.zeros((npad,), np.float32)
    s_pad[:N] = s
    y_pad = np.zeros((npad,), np.float32)
    y_pad[:N] = y
    q = np.clip(np.floor(y.astype(np.float64) * QSY).astype(np.int64),
                0, QSY - 1)
    qy1_pad = np.full((npad,), -1.0, np.float32)
    qy1_pad[:N] = (q // C2Y).astype(np.float32)
    qy2_pad = np.full((npad,), -1.0, np.float32)
    qy2_pad[:N] = (q % C2Y).astype(np.float32)
    qs = np.clip(np.floor((s.astype(np.float64) - LO) / DELTA).astype(
        np.int64), 0, MBS - 1)
    qs1_pad = np.full((npad,), -1.0, np.float32)
    qs1_pad[:N] = (qs // C2S).astype(np.float32)
    qs2_pad = np.full((npad,), -1.0, np.float32)
    qs2_pad[:N] = (qs % C2S).astype(np.float32)
    cpack, negio = _consts()
    pp = np.arange(128)
    wsel = (_OMEGA[pp // 4][None, :]
            * (np.arange(4)[:, None] == pp[None, :] % 4)).astype(np.float32)
    maps = []
    for d in range(NCORES):
        sl = slice(d * PB, (d + 1) * PB)
        sv = s_pad[sl]
        s_row2 = np.concatenate([sv.reshape(4, QB), wsel],
                                axis=1).astype(np.float32)
        # per-block lhsT layout [NB+1, 5*128]: qT[b, g*128+p] = value of
        # item b*128+p in group g; row NB = ones
        qT = np.ones((NB + 1, 5 * 128), np.float32)
        qT[0:NB, 0:128] = qy1_pad[sl].reshape(NB, 128)
        qT[0:NB, 128:256] = qy2_pad[sl].reshape(NB, 128)
        qT[0:NB, 256:384] = qs1_pad[sl].reshape(NB, 128)
        qT[0:NB, 384:512] = qs2_pad[sl].reshape(NB, 128)
        qT[0:NB, 512:640] = y_pad[sl].reshape(NB, 128)
        maps.append({
            "qT": np.ascontiguousarray(qT).astype(ml_dtypes.bfloat16),
            "negio": negio,
            "s_row2": np.ascontiguousarray(s_row2),
            "cpack": cpack,
        })
    return maps


def kernel(logits, targets):
    nc = _get_nc()
    res = run_bass_kernel_spmd(nc, _in_maps(logits, targets),
                               core_ids=list(range(NCORES)))
    out = np.asarray(res.results[0]["out"], dtype=np.float32)
    return out.reshape(())


# revision 24
# speedup vs baseline: 1.1003x; 1.0451x over previous
"""ApproxNDCGLoss on 8 TRN2 NeuronCores (Bass/Tile) — one-collective design.

loss = 1 - dcg/(idcg+1e-8):
  approx_rank[j] = 1 + sum_i sigmoid(s[j]-s[i])
  dcg  = sum_j y[j] / log2(approx_rank[j]+1)
  idcg = sum_j y[j] / log2(rank_y[j]+1),  rank_y[j] = 1 + #{i: y[i] > y[j]}

Everything per-item is folded into per-BUCKET sums that are additive across
cores, so a single fused AllReduce replaces the old (T-table AR + per-item
lookup + partials AR) pipeline:

DCG:  sigmoid(x) - 1/2 ~= sum_k b_k sin(w_k x)  (K=32 sine series), so
  rank(t)+1 = n/2 + 2 + sum_k b_k [sin(w_k t) C_k - cos(w_k t) S_k]
  with C/S = global trig sums.  Scores are binned into 1024 buckets
  (32x32 two-level); per-bucket y-sums Ys are AllReduced, and post-AR the
  series is evaluated at all bucket centers with ONE 64-contraction matmul
  via the angle-addition split  theta = A(c1) + B(c2):
      ser[c1,c2] = sum_k U_k(c1) cosB_k(c2) + V_k(c1) sinB_k(c2)
  where U,V are [32,32] tiles built from (C,S) and host trig constants.
  dcg = sum_b Ys_b * ln2/ln(ser_b + n/2 + 2).  Bucketing error ~1e-5.

IDCG: y in [0,1) binned into 2048 buckets (64x32).  Joint histogram via
  one-hot matmuls; suffix-count table T (strict suffix + hist/2) is linear
  in hist so each core builds its local T pre-collective.  Per-bucket
  y-sums Ysum give  idcg = sum_b Ysum_b * ln2/ln(T_b + 1.5).

The collectives share DMA bandwidth with input loads across all 8 cores,
so the mesh start is gated by the slowest core's (launch stagger + local
compute), and input bytes burn shared DMA time.  Only ~220KB/core is
shipped: per-block ids/weights qT [21,640], raw quad scores [4,768], and
packed constants [128,352].  On device, PE outer-product matmuls against
an iota-built block-selector expand ids to (q - class) differences in one
shot (an extra ones-row in lhsT folds the -iota right into the matmul), a
single IS_EQ-with-0 then yields each bf16 one-hot; w_k*s comes from a
4-contraction matmul against an omega-selector.  The y-expansion is built
once at 32 wide and shared by both weighted histograms.  No partition
broadcasts, no per-item post-AR work, fat DMA descriptors only.
"""

import numpy as np
import ml_dtypes

import concourse.bacc as bacc
import concourse.bass as bass
import concourse.mybir as mybir
import concourse.tile as tile
from concourse.bass_utils import run_bass_kernel_spmd
from concourse.tile_rust import add_dep_helper

N = 20000
NCORES = 8
PB = 2560                   # items per core (padded; 8*2560 = 20480)
NB = PB // 128              # 20 column blocks of 128 items
QB = PB // 4                # 640 free elems in the quad trig layout
HQ = QB // 2                # trig processed in two 320-wide PSUM halves
K = 32                      # Fourier terms
L = 24.2                    # period of the sine series
TRIG_PAD = NCORES * PB - N  # 480 zero-score pads -> C_k -= 480
# y buckets: 2048 = 64 (partitions) x 32 (free)
QSY = 2048
C1Y = 64
C2Y = 32
W64 = NB * C1Y              # 1280 (level-1 one-hot width)
W32 = NB * C2Y              # 640  (level-2 / 32-class one-hot width)
# score buckets: 1024 = 32 x 32
MBS = 1024
C1S = 32
C2S = 32
LO, HI = -5.5, 5.5
DELTA = (HI - LO) / MBS
LN2 = float(np.log(2.0))

_B = np.array([
    0.575840175151825, -0.0012469458160921931, 0.08171718567609787,
    0.019092485308647156, -0.007231124211102724, 0.02490580640733242,
    -0.017197489738464355, 0.014312449842691422, -0.007428332697600126,
    0.003442077897489071, -0.0007101596565917134, 3.444465983193368e-05,
    -0.00029458850622177124, 0.0009411321370862424, -0.0013493510195985436,
    0.0013473577564582229, -0.0009938474977388978, 0.0005221660248935223,
    -0.00015226299001369625, 2.9422192255879054e-06, -5.903289275011048e-05,
    0.00021578818268608302, -0.0003499265294522047, 0.0003830934874713421,
    -0.00030826698639430106, 0.0001763014297466725, -5.747509567299858e-05,
    2.007998773478903e-06, -1.8746375644695945e-05, 7.875602022977546e-05,
    -0.00013714544184040278, 0.00015883310697972775], dtype=np.float32)
_OMEGA = (2.0 * np.pi * np.arange(1, K + 1) / L).astype(np.float32)

# range reduction: m = x - round(x/2pi)*2pi via magic-number round and a
# 3-term Cody-Waite cascade.
_MAGIC = float(np.float32(1.5 * 2.0 ** 23))
_INV2PI = float(np.float32(1.0 / (2.0 * np.pi)))
_CW1 = 6.28125
_CW2 = float(np.float32(2.0 * np.pi - 6.28125))
_CW3 = float(np.float32(2.0 * np.pi - 6.28125
                        - np.float64(np.float32(2.0 * np.pi - 6.28125))))
_PI = float(np.pi)

_CACHE = {}


def _build():
    f32 = mybir.dt.float32
    bf16 = mybir.dt.bfloat16
    AF = mybir.ActivationFunctionType
    ALU = mybir.AluOpType
    X = mybir.AxisListType.X

    nc = bacc.Bacc("TRN2", target_bir_lowering=False, debug=False,
                   num_devices=NCORES)
    # qT col-groups of 128: q1y | q2y | qs1 | qs2 | y.  Row 20 = ones (the
    # -iota row of the rep operand turns the expansion into q - class).
    qt_dram = nc.dram_tensor("qT", [NB + 1, 5 * 128], bf16,
                             kind="ExternalInput")
    # negio: cols 0:W64 = -(iota64 tiled), W64:W64+W32 = -(iota32 tiled)
    ng_dram = nc.dram_tensor("negio", [1, W64 + W32], bf16,
                             kind="ExternalInput")
    # s_row2: quad scores [4, 640] | omega-selector wsel [4, 128]
    sr_dram = nc.dram_tensor("s_row2", [4, QB + 128], f32,
                             kind="ExternalInput")
    # cpack cols: selK[128,32] | trS[64,64] | trH[32,32] | id64[64,64] |
    #             uvc[32,128] | cBsB[64,32]
    cp_dram = nc.dram_tensor("cpack", [128, 352], f32, kind="ExternalInput")
    out_dram = nc.dram_tensor("out", [1, 1], f32, kind="ExternalOutput")

    groups = [list(range(NCORES))]

    with tile.TileContext(nc) as tc:
        with tc.tile_pool(name="sbuf", bufs=1) as pool, \
             tc.tile_pool(name="psum", bufs=1, space="PSUM") as psum, \
             tc.tile_pool(name="dram", bufs=1, space="DRAM") as dram:
            # ---------- block-selector iotas (gpsimd, no deps) ----------
            ior64 = pool.tile([NB, W64], f32)
            nc.gpsimd.iota(ior64[:].rearrange("p (b c) -> p b c", c=C1Y),
                           pattern=[[1, NB], [0, C1Y]], base=0,
                           channel_multiplier=-1,
                           allow_small_or_imprecise_dtypes=True)
            ior32 = pool.tile([NB, W32], f32)
            nc.gpsimd.iota(ior32[:].rearrange("p (b c) -> p b c", c=C2Y),
                           pattern=[[1, NB], [0, C2Y]], base=0,
                           channel_multiplier=-1,
                           allow_small_or_imprecise_dtypes=True)

            # ---------- input loads (sync/scalar queues only) ----------
            qT = pool.tile([NB + 1, 5 * 128], bf16)
            nc.sync.dma_start(qT[:], qt_dram[:])
            negio = pool.tile([1, W64 + W32], bf16)
            nc.sync.dma_start(negio[:], ng_dram[:])
            s_row = pool.tile([4, QB + 128], f32)
            nc.scalar.dma_start(s_row[:], sr_dram[:])
            cpack = pool.tile([128, 352], f32)
            qeng = [nc.sync, nc.scalar]
            for i in range(4):
                sl = slice(i * 32, (i + 1) * 32)
                qeng[i % 2].dma_start(cpack[sl, :], cp_dram[sl, :])
            selK = cpack[:, 0:32]
            trS = cpack[0:64, 32:96]
            trH = cpack[0:C2Y, 96:128]
            id64 = cpack[0:64, 128:192]
            uvc = cpack[0:K, 192:320]
            cBsB = cpack[0:2 * K, 320:352]

            # rep operands: rows 0:NB one-hot block selector, row NB = -iota
            rep64 = pool.tile([NB + 1, W64], bf16)
            nc.vector.tensor_scalar(rep64[0:NB, :], ior64[:], 0.0, None,
                                    ALU.is_equal)
            nc.sync.dma_start(rep64[NB:NB + 1, :], negio[:, 0:W64])
            rep32 = pool.tile([NB + 1, W32], bf16)
            nc.vector.tensor_scalar(rep32[0:NB, :], ior32[:], 0.0, None,
                                    ALU.is_equal)
            nc.scalar.dma_start(rep32[NB:NB + 1, :], negio[:, W64:])

            ones1 = pool.tile([1, 1], f32)
            nc.vector.memset(ones1[:], 1.0)
            lnb1 = pool.tile([1, 1], f32)
            nc.vector.memset(lnb1[:], 1.0)

            # ---------- trig features: s_w = wsel @ s_row, then series ----
            sparts = pool.tile([128, 2], f32)
            nc.vector.memset(sparts[:], 0.0)
            cos_ins = None
            cs_ps = psum.tile([K, 2], f32, tag="pcs", bufs=1)
            for h in range(2):
                hs = slice(h * HQ, (h + 1) * HQ)
                pwt = psum.tile([128, 512], f32, tag="pA", bufs=2)
                pw = pwt[:, 0:HQ]
                nc.tensor.matmul(pw, lhsT=s_row[0:4, QB:QB + 128],
                                 rhs=s_row[0:4, hs], start=True, stop=True)
                rnd = pool.tile([128, HQ], f32, tag="rnd", bufs=2)
                nc.scalar.activation(rnd[:], pw, AF.Copy, bias=_MAGIC,
                                     scale=_INV2PI)
                kint = pool.tile([128, HQ], f32, tag="kint", bufs=2)
                nc.vector.tensor_scalar(kint[:], rnd[:], _MAGIC, None,
                                        ALU.subtract)
                sa = pool.tile([128, HQ], f32, tag="sa", bufs=2)
                nc.vector.cody_waite_cascade(sa[:], pw, kint[:],
                                             _CW1, _CW2, _CW3)
                clamp = float(np.float32(_PI))
                nc.vector.tensor_scalar(sa[:], sa[:], clamp, -clamp,
                                        ALU.min, ALU.max)
                ca = pool.tile([128, HQ], f32, tag="ca", bufs=2)
                nc.vector.add_range_wrap(ca[:], sa[:], _PI / 2, _PI, 2 * _PI)
                nc.vector.tensor_scalar(ca[:], ca[:], clamp, -clamp,
                                        ALU.min, ALU.max)
                sin_t = pool.tile([128, HQ], f32, tag="sint", bufs=2)
                nc.scalar.activation(sin_t[:], sa[:], AF.Sin,
                                     accum_out=sparts[:, 0:1])
                cos_t = pool.tile([128, HQ], f32, tag="cost", bufs=2)
                cos_ins = nc.scalar.activation(cos_t[:], ca[:], AF.Sin,
                                               accum_out=sparts[:, 1:2])
                nc.tensor.matmul(cs_ps[:], lhsT=selK, rhs=sparts[:],
                                 start=(h == 0), stop=(h == 1),
                                 skip_group_check=True)
            # switch the ACT table to Ln now; post-AR Ln finds it loaded
            lnwarm = pool.tile([1, 1], f32)
            warm_ins = nc.scalar.activation(lnwarm[:], ones1[:], AF.Ln,
                                            bias=lnb1[:])
            add_dep_helper(warm_ins.ins, cos_ins.ins, False,
                           "Ln table load after the Sin stream")
            cs_sb = pool.tile([K, 2], f32)
            nc.scalar.copy(cs_sb[:], cs_ps[:])

            # ---------- expansions (q - class into PSUM) + one-hots -------
            h1y = pool.tile([128, W64], bf16)
            rhs_y = pool.tile([128, 2 * W32], bf16)
            yx = pool.tile([128, W32], bf16)
            h1s = pool.tile([128, W32], bf16)
            h2se = pool.tile([128, W32], bf16)
            h2sw = pool.tile([128, W32], bf16)

            # q1y: 64-class, 3 chunks
            for c0, c1 in ((0, 512), (512, 1024), (1024, W64)):
                pe = psum.tile([128, 512], f32, tag="pA", bufs=2)
                nc.tensor.matmul(pe[:, 0:c1 - c0], lhsT=qT[:, 0:128],
                                 rhs=rep64[:, c0:c1], start=True, stop=True,
                                 skip_group_check=True)
                nc.vector.tensor_scalar(h1y[:, c0:c1], pe[:, 0:c1 - c0],
                                        0.0, None, ALU.is_equal)
            # q2y + shared y expansion + weighted half, 2 chunks of 320
            for c0 in (0, 320):
                c1 = c0 + 320
                pe = psum.tile([128, 512], f32, tag="pA", bufs=2)
                nc.tensor.matmul(pe[:, 0:320], lhsT=qT[:, 128:256],
                                 rhs=rep32[:, c0:c1], start=True, stop=True,
                                 skip_group_check=True)
                nc.vector.tensor_scalar(rhs_y[:, c0:c1], pe[:, 0:320],
                                        0.0, None, ALU.is_equal)
                pe2 = psum.tile([128, 512], f32, tag="pA", bufs=2)
                nc.tensor.matmul(pe2[:, 0:320], lhsT=qT[0:NB, 512:640],
                                 rhs=rep32[0:NB, c0:c1], start=True,
                                 stop=True, skip_group_check=True)
                nc.vector.tensor_scalar(yx[:, c0:c1], pe2[:, 0:320], 1.0,
                                        None, ALU.mult)
                nc.vector.tensor_tensor(rhs_y[:, W32 + c0:W32 + c1],
                                        rhs_y[:, c0:c1], yx[:, c0:c1],
                                        ALU.mult)
            # qs1 / qs2, 2 chunks each; weighted s-side on gpsimd
            for c0 in (0, 320):
                c1 = c0 + 320
                pe = psum.tile([128, 512], f32, tag="pA", bufs=2)
                nc.tensor.matmul(pe[:, 0:320], lhsT=qT[:, 256:384],
                                 rhs=rep32[:, c0:c1], start=True, stop=True,
                                 skip_group_check=True)
                nc.vector.tensor_scalar(h1s[:, c0:c1], pe[:, 0:320],
                                        0.0, None, ALU.is_equal)
                pe2 = psum.tile([128, 512], f32, tag="pA", bufs=2)
                nc.tensor.matmul(pe2[:, 0:320], lhsT=qT[:, 384:512],
                                 rhs=rep32[:, c0:c1], start=True, stop=True,
                                 skip_group_check=True)
                nc.vector.tensor_scalar(h2se[:, c0:c1], pe2[:, 0:320],
                                        0.0, None, ALU.is_equal)
                nc.gpsimd.tensor_tensor(h2sw[:, c0:c1], h2se[:, c0:c1],
                                        yx[:, c0:c1], ALU.mult)

            # ---------- weighted histograms (2 interleaved groups) --------
            psY = psum.tile([C1Y, 2 * C2Y], f32, tag="py", bufs=1)
            psSt = psum.tile([64, 96], f32, tag="pC", bufs=2)
            psS = psSt[0:C1S, 0:C2S]
            rhs_yv = rhs_y[:].rearrange("p (h x) -> p h x", h=2)
            for b in range(NB):
                nc.tensor.matmul(
                    psY[:], lhsT=h1y[:, b * C1Y:(b + 1) * C1Y],
                    rhs=rhs_yv[:, :, b * C2Y:(b + 1) * C2Y],
                    start=(b == 0), stop=(b == NB - 1),
                    skip_group_check=True)
                nc.tensor.matmul(
                    psS, lhsT=h1s[:, b * C1S:(b + 1) * C1S],
                    rhs=h2sw[:, b * C2S:(b + 1) * C2S],
                    start=(b == 0), stop=(b == NB - 1),
                    skip_group_check=True)

            # ---------- local suffix table T ----------
            hist_sb = pool.tile([C1Y, C2Y], f32)
            nc.scalar.copy(hist_sb[:], psY[:, 0:C2Y])
            ysum_sb = pool.tile([C1Y, C2Y], f32)
            nc.scalar.copy(ysum_sb[:], psY[:, C2Y:2 * C2Y])
            ys_sb = pool.tile([C1S, C2S], f32)
            nc.scalar.copy(ys_sb[:], psS)
            rowsum = pool.tile([C1Y, 1], f32)
            nc.vector.tensor_reduce(rowsum[:], psY[:, 0:C2Y], axis=X,
                                    op=ALU.add)
            htpt = psum.tile([64, 96], f32, tag="pC", bufs=2)
            htp = htpt[0:C2Y, 0:C1Y]
            nc.tensor.transpose(htp, hist_sb[:], id64)
            hts = pool.tile([C2Y, C1Y], f32)
            nc.scalar.copy(hts[:], htp)
            srfx = psum.tile([64, 96], f32, tag="pC", bufs=2)
            nc.tensor.matmul(srfx[:, 0:C2Y], lhsT=hts[:], rhs=trH,
                             start=True, stop=True, skip_group_check=True)
            nc.tensor.matmul(srfx[:, 64:65], lhsT=trS, rhs=rowsum[:],
                             start=True, stop=True, skip_group_check=True)
            t_loc = pool.tile([C1Y, C2Y], f32)
            nc.vector.tensor_scalar(t_loc[:], srfx[:, 0:C2Y],
                                    srfx[:, 64:65], None, ALU.add)

            # ---------- fused AllReduce: [T | Ysum | Ys | C/S] ----------
            PWC = C2Y + C2Y + C2S + 2          # 98 payload cols
            cc_in = dram.tile([C1Y, PWC], f32)
            cc_out = dram.tile([C1Y, PWC], f32, addr_space="Shared")
            zfill = pool.tile([C1Y - C1S, C2S + 2], f32)
            nc.vector.memset(zfill[:], 0.0)
            nc.scalar.dma_start(cc_in[C1S:C1Y, 2 * C2Y:PWC], zfill[:])
            nc.sync.dma_start(cc_in[:, 0:C2Y], t_loc[:])
            nc.sync.dma_start(cc_in[:, C2Y:2 * C2Y], ysum_sb[:])
            nc.scalar.dma_start(cc_in[0:C1S, 2 * C2Y:2 * C2Y + C2S],
                                ys_sb[:])
            nc.scalar.dma_start(cc_in[0:K, 2 * C2Y + C2S:PWC], cs_sb[:])
            nc.gpsimd.collective_compute(
                "AllReduce", ALU.add, replica_groups=groups,
                ins=[cc_in[:, :].opt()], outs=[cc_out[:, :].opt()])
            t_glob = pool.tile([C1Y, C2Y], f32)
            nc.sync.dma_start(t_glob[:], cc_out[:, 0:C2Y])
            ysg = pool.tile([C1Y, C2Y], f32)
            nc.sync.dma_start(ysg[:], cc_out[:, C2Y:2 * C2Y])
            yss = pool.tile([C1S, C2S], f32)
            nc.scalar.dma_start(yss[:], cc_out[0:C1S, 2 * C2Y:2 * C2Y + C2S])
            csg = pool.tile([K, 2], f32)
            nc.scalar.dma_start(csg[:], cc_out[0:K, 2 * C2Y + C2S:PWC])

            # ---------- dcg: series at score-bucket centers ----------
            # csg col0 = S_k, col1 = C_k; pads contribute cos(0)=1 each
            nc.vector.tensor_scalar(csg[:, 1:2], csg[:, 1:2],
                                    float(TRIG_PAD), None, ALU.subtract)
            # luv rows 0:K = -U, K:2K = -V  (negated; Ln uses scale=-1)
            luv = pool.tile([2 * K, C1S], f32)
            u1 = pool.tile([K, C1S], f32)
            nc.vector.tensor_scalar(u1[:], uvc[:, 0:32], csg[:, 1:2], None,
                                    ALU.mult)
            nc.vector.scalar_tensor_tensor(luv[0:K, :], uvc[:, 32:64],
                                           csg[:, 0:1], u1[:],
                                           ALU.mult, ALU.subtract)
            v1 = pool.tile([K, C1S], f32)
            nc.vector.tensor_scalar(v1[:], uvc[:, 96:128], csg[:, 1:2],
                                    None, ALU.mult)
            nc.vector.scalar_tensor_tensor(luv[K:2 * K, :], uvc[:, 64:96],
                                           csg[:, 0:1], v1[:],
                                           ALU.mult, ALU.add)
            rank_pst = psum.tile([64, 96], f32, tag="pC", bufs=2)
            rank_ps = rank_pst[0:C1S, 0:C2S]
            nc.tensor.matmul(rank_ps, lhsT=luv[:], rhs=cBsB,
                             start=True, stop=True)
            dbias = pool.tile([C1S, 1], f32)
            nc.vector.memset(dbias[:], float(N / 2 + 2.0))
            ibias = pool.tile([C1Y, 1], f32)
            nc.vector.memset(ibias[:], 1.5)
            parts = pool.tile([64, 3], f32)
            nc.vector.memset(parts[:], 0.0)
            lnds = pool.tile([C1S, C2S], f32)
            nc.scalar.activation(lnds[:], rank_ps, AF.Ln,
                                 bias=dbias[:], scale=-1.0)
            rds = pool.tile([C1S, C2S], f32)
            nc.vector.reciprocal(rds[:], lnds[:])
            scrD = pool.tile([C1S, C2S], f32)
            nc.vector.scalar_tensor_tensor(scrD[:], yss[:], LN2, rds[:],
                                           ALU.mult, ALU.mult,
                                           accum_out=parts[0:C1S, 0:1])
            # ---------- idcg: per-bucket mid-rank discount ----------
            lnis = pool.tile([C1Y, C2Y], f32)
            nc.scalar.activation(lnis[:], t_glob[:], AF.Ln, bias=ibias[:])
            ris = pool.tile([C1Y, C2Y], f32)
            nc.vector.reciprocal(ris[:], lnis[:])
            scrI = pool.tile([C1Y, C2Y], f32)
            nc.vector.scalar_tensor_tensor(scrI[:], ysg[:], LN2, ris[:],
                                           ALU.mult, ALU.mult,
                                           accum_out=parts[:, 1:2])
            nc.vector.tensor_reduce(parts[:, 2:3], ysg[:], axis=X,
                                    op=ALU.add)

            # ---------- fold partitions, final scalar loss ----------
            ones64 = pool.tile([64, 1], f32)
            nc.vector.memset(ones64[:], 1.0)
            ps2t = psum.tile([64, 96], f32, tag="pC", bufs=2)
            ps2 = ps2t[0:1, 0:3]
            nc.tensor.matmul(ps2, lhsT=ones64[:], rhs=parts[:],
                             start=True, stop=True)
            d1 = pool.tile([1, 1], f32)
            nc.vector.tensor_scalar(d1[:], ps2t[0:1, 1:2], 1e-8, None,
                                    ALU.add)
            rec = pool.tile([1, 1], f32)
            nc.vector.reciprocal(rec[:], d1[:])
            negl = pool.tile([1, 1], f32)
            nc.vector.scalar_tensor_tensor(negl[:], ps2t[0:1, 0:1], rec[:],
                                           ones1[:], ALU.mult, ALU.subtract)
            negm = pool.tile([1, 1], f32)
            nc.vector.tensor_scalar(negm[:], ps2t[0:1, 2:3], 1.0, -1.0,
                                    ALU.is_ge, ALU.mult)
            fin = pool.tile([1, 1], f32)
            nc.vector.tensor_tensor(fin[:], negl[:], negm[:], ALU.mult)
            nc.sync.dma_start(out_dram[:], fin[:])

    nc.compile()
    return nc


def _get_nc():
    if "nc" not in _CACHE:
        _CACHE["nc"] = _build()
    return _CACHE["nc"]


def _consts():
    p = np.arange(128)
    selK = (p[:, None] // 4 == np.arange(K)[None, :]).astype(np.float32)
    a = np.arange(64)
    trS = (a[:, None] > a[None, :]).astype(np.float32)
    a2 = np.arange(C2Y)
    trH = ((a2[:, None] > a2[None, :]).astype(np.float32)
           + 0.5 * (a2[:, None] == a2[None, :]).astype(np.float32))
    id64 = np.eye(64, dtype=np.float32)
    om = _OMEGA.astype(np.float64)[:, None]
    aang = om * (LO + np.arange(C1S, dtype=np.float64)[None, :]
                 * C2S * DELTA)
    bang = om * ((np.arange(C2S, dtype=np.float64)[None, :] + 0.5) * DELTA)
    bk = _B.astype(np.float64)[:, None]
    sAb = (bk * np.sin(aang)).astype(np.float32)
    cAb = (bk * np.cos(aang)).astype(np.float32)
    # device layout: 0:32 sAb, 32:64 cAb, 64:96 -sAb, 96:128 -cAb
    uvc = np.concatenate([sAb, cAb, -sAb, -cAb], axis=1)
    cBsB = np.concatenate([np.cos(bang), np.sin(bang)],
                          axis=0).astype(np.float32)
    cpack = np.zeros((128, 352), np.float32)
    cpack[:, 0:32] = selK
    cpack[0:64, 32:96] = trS
    cpack[0:C2Y, 96:128] = trH
    cpack[0:64, 128:192] = id64
    cpack[0:K, 192:320] = uvc
    cpack[0:2 * K, 320:352] = cBsB
    negio = np.concatenate([
        -np.tile(np.arange(C1Y, dtype=np.float32), NB),
        -np.tile(np.arange(C2Y, dtype=np.float32), NB)]).reshape(1, -1)
    return np.ascontiguousarray(cpack), \
        np.ascontiguousarray(negio).astype(ml_dtypes.bfloat16)


def _in_maps(logits, targets):
    s = np.asarray(logits, dtype=np.float32).reshape(-1)
    y = np.asarray(targets, dtype=np.float32).reshape(-1)
    npad = NCORES * PB
    s_pad = np.zeros((npad,), np.float32)
    s_pad[:N] = s
    y_pad = np.zeros((npad,), np.float32)
    y_pad[:N] = y
    q = np.clip(np.floor(y.astype(np.float64) * QSY).astype(np.int64),
                0, QSY - 1)
    qy1_pad = np.full((npad,), -1.0, np.float32)
    qy1_pad[:N] = (q // C2Y).astype(np.float32)
    qy2_pad = np.full((npad,), -1.0, np.float32)
    qy2_pad[:N] = (q % C2Y).astype(np.float32)
    qs = np.clip(np.floor((s.astype(np.float64) - LO) / DELTA).astype(
        np.int64), 0, MBS - 1)
    qs1_pad = np.full((npad,), -1.0, np.float32)
    qs1_pad[:N] = (qs // C2S).astype(np.float32)
    qs2_pad = np.full((npad,), -1.0, np.float32)
    qs2_pad[:N] = (qs % C2S).astype(np.float32)
    cpack, negio = _consts()
    pp = np.arange(128)
    wsel = (_OMEGA[pp // 4][None, :]
            * (np.arange(4)[:, None] == pp[None, :] % 4)).astype(np.float32)
    maps = []
    for d in range(NCORES):
        sl = slice(d * PB, (d + 1) * PB)
        sv = s_pad[sl]
        s_row2 = np.concatenate([sv.reshape(4, QB), wsel],
                                axis=1).astype(np.float32)
        # per-block lhsT layout [NB+1, 5*128]: qT[b, g*128+p] = value of
        # item b*128+p in group g; row NB = ones
        qT = np.ones((NB + 1, 5 * 128), np.float32)
        qT[0:NB, 0:128] = qy1_pad[sl].reshape(NB, 128)
        qT[0:NB, 128:256] = qy2_pad[sl].reshape(NB, 128)
        qT[0:NB, 256:384] = qs1_pad[sl].reshape(NB, 128)
        qT[0:NB, 384:512] = qs2_pad[sl].reshape(NB, 128)
        qT[0:NB, 512:640] = y_pad[sl].reshape(NB, 128)
        maps.append({
            "qT": np.ascontiguousarray(qT).astype(ml_dtypes.bfloat16),
            "negio": negio,
            "s_row2": np.ascontiguousarray(s_row2),
            "cpack": cpack,
        })
    return maps


def kernel(logits, targets):
    nc = _get_nc()
    res = run_bass_kernel_spmd(nc, _in_maps(logits, targets),
                               core_ids=list(range(NCORES)))
    out = np.asarray(res.results[0]["out"], dtype=np.float32)
    return out.reshape(())


# revision 25
# speedup vs baseline: 1.2338x; 1.1213x over previous
"""ApproxNDCGLoss on 8 TRN2 NeuronCores (Bass/Tile) — one-collective design.

loss = 1 - dcg/(idcg+1e-8):
  approx_rank[j] = 1 + sum_i sigmoid(s[j]-s[i])
  dcg  = sum_j y[j] / log2(approx_rank[j]+1)
  idcg = sum_j y[j] / log2(rank_y[j]+1),  rank_y[j] = 1 + #{i: y[i] > y[j]}

Everything per-item is folded into per-BUCKET sums that are additive across
cores, so a single fused AllReduce replaces the old (T-table AR + per-item
lookup + partials AR) pipeline:

DCG:  sigmoid(x) - 1/2 ~= sum_k b_k sin(w_k x)  (K=32 sine series), so
  rank(t)+1 = n/2 + 2 + sum_k b_k [sin(w_k t) C_k - cos(w_k t) S_k]
  with C/S = global trig sums.  Scores are binned into 1024 buckets
  (32x32 two-level); per-bucket y-sums Ys are AllReduced, and post-AR the
  series is evaluated at all bucket centers with ONE 64-contraction matmul
  via the angle-addition split  theta = A(c1) + B(c2):
      ser[c1,c2] = sum_k U_k(c1) cosB_k(c2) + V_k(c1) sinB_k(c2)
  where U,V are [32,32] tiles built from (C,S) and host trig constants.
  dcg = sum_b Ys_b * ln2/ln(ser_b + n/2 + 2).  Bucketing error ~1e-5.

IDCG: y in [0,1) binned into 2048 buckets (64x32).  Joint histogram via
  one-hot matmuls; suffix-count table T (strict suffix + hist/2) is linear
  in hist so each core builds its local T pre-collective.  Per-bucket
  y-sums Ysum give  idcg = sum_b Ysum_b * ln2/ln(T_b + 1.5).

The collectives share DMA bandwidth with input loads across all 8 cores,
so the mesh start is gated by the slowest core's (launch stagger + local
compute), and input bytes burn shared DMA time.  Only ~220KB/core is
shipped: per-block ids/weights qT [21,640], raw quad scores [4,768], and
packed constants [128,352].  On device, PE outer-product matmuls against
an iota-built block-selector expand ids to (q - class) differences in one
shot (an extra ones-row in lhsT folds the -iota right into the matmul), a
single IS_EQ-with-0 then yields each bf16 one-hot; w_k*s comes from a
4-contraction matmul against an omega-selector.  The y-expansion is built
once at 32 wide and shared by both weighted histograms.  No partition
broadcasts, no per-item post-AR work, fat DMA descriptors only.
"""

import numpy as np
import ml_dtypes

import concourse.bacc as bacc
import concourse.bass as bass
import concourse.mybir as mybir
import concourse.tile as tile
from concourse.bass_utils import run_bass_kernel_spmd
from concourse.tile_rust import add_dep_helper

N = 20000
NCORES = 8
PB = 2560                   # items per core (padded; 8*2560 = 20480)
NB = PB // 128              # 20 column blocks of 128 items
QB = PB // 4                # 640 free elems in the quad trig layout
HQ = QB // 2                # trig processed in two 320-wide PSUM halves
K = 32                      # Fourier terms
L = 24.2                    # period of the sine series
TRIG_PAD = NCORES * PB - N  # 480 zero-score pads -> C_k -= 480
# y buckets: 2048 = 64 (partitions) x 32 (free)
QSY = 2048
C1Y = 64
C2Y = 32
W64 = NB * C1Y              # 1280 (level-1 one-hot width)
W32 = NB * C2Y              # 640  (level-2 / 32-class one-hot width)
# score buckets: 1024 = 32 x 32
MBS = 1024
C1S = 32
C2S = 32
LO, HI = -5.5, 5.5
DELTA = (HI - LO) / MBS
LN2 = float(np.log(2.0))

_B = np.array([
    0.575840175151825, -0.0012469458160921931, 0.08171718567609787,
    0.019092485308647156, -0.007231124211102724, 0.02490580640733242,
    -0.017197489738464355, 0.014312449842691422, -0.007428332697600126,
    0.003442077897489071, -0.0007101596565917134, 3.444465983193368e-05,
    -0.00029458850622177124, 0.0009411321370862424, -0.0013493510195985436,
    0.0013473577564582229, -0.0009938474977388978, 0.0005221660248935223,
    -0.00015226299001369625, 2.9422192255879054e-06, -5.903289275011048e-05,
    0.00021578818268608302, -0.0003499265294522047, 0.0003830934874713421,
    -0.00030826698639430106, 0.0001763014297466725, -5.747509567299858e-05,
    2.007998773478903e-06, -1.8746375644695945e-05, 7.875602022977546e-05,
    -0.00013714544184040278, 0.00015883310697972775], dtype=np.float32)
_OMEGA = (2.0 * np.pi * np.arange(1, K + 1) / L).astype(np.float32)

# range reduction: m = x - round(x/2pi)*2pi via magic-number round and a
# 3-term Cody-Waite cascade.
_MAGIC = float(np.float32(1.5 * 2.0 ** 23))
_INV2PI = float(np.float32(1.0 / (2.0 * np.pi)))
_CW1 = 6.28125
_CW2 = float(np.float32(2.0 * np.pi - 6.28125))
_CW3 = float(np.float32(2.0 * np.pi - 6.28125
                        - np.float64(np.float32(2.0 * np.pi - 6.28125))))
_PI = float(np.pi)

_CACHE = {}


def _build():
    f32 = mybir.dt.float32
    bf16 = mybir.dt.bfloat16
    AF = mybir.ActivationFunctionType
    ALU = mybir.AluOpType
    X = mybir.AxisListType.X

    nc = bacc.Bacc("TRN2", target_bir_lowering=False, debug=False,
                   num_devices=NCORES)
    # qT col-groups of 128: q1y | q2y | qs1 | qs2 | y.  Row 20 = ones (the
    # -iota row of the rep operand turns the expansion into q - class).
    qt_dram = nc.dram_tensor("qT", [NB + 1, 5 * 128], bf16,
                             kind="ExternalInput")
    # negio: cols 0:W64 = -(iota64 tiled), W64:W64+W32 = -(iota32 tiled)
    ng_dram = nc.dram_tensor("negio", [1, W64 + W32], bf16,
                             kind="ExternalInput")
    # s_row2: quad scores [4, 640] | omega-selector wsel [4, 128]
    sr_dram = nc.dram_tensor("s_row2", [4, QB + 128], f32,
                             kind="ExternalInput")
    # cpack cols: selK[128,32] | trS[64,64] | trH[32,32] | id64[64,64] |
    #             uvc[32,192] | cBsB[64,32]
    cp_dram = nc.dram_tensor("cpack", [128, 416], f32, kind="ExternalInput")
    out_dram = nc.dram_tensor("out", [1, 1], f32, kind="ExternalOutput")

    groups = [list(range(NCORES))]

    with tile.TileContext(nc) as tc:
        with tc.tile_pool(name="sbuf", bufs=1) as pool, \
             tc.tile_pool(name="psum", bufs=1, space="PSUM") as psum, \
             tc.tile_pool(name="dram", bufs=1, space="DRAM") as dram:
            # ---------- block-selector iotas (gpsimd, no deps) ----------
            ior64 = pool.tile([NB, W64], f32)
            nc.gpsimd.iota(ior64[:].rearrange("p (b c) -> p b c", c=C1Y),
                           pattern=[[1, NB], [0, C1Y]], base=0,
                           channel_multiplier=-1,
                           allow_small_or_imprecise_dtypes=True)
            ior32 = pool.tile([NB, W32], f32)
            nc.gpsimd.iota(ior32[:].rearrange("p (b c) -> p b c", c=C2Y),
                           pattern=[[1, NB], [0, C2Y]], base=0,
                           channel_multiplier=-1,
                           allow_small_or_imprecise_dtypes=True)

            # ---------- input loads (sync/scalar queues only) ----------
            qT = pool.tile([NB + 1, 5 * 128], bf16)
            nc.sync.dma_start(qT[:], qt_dram[:])
            negio = pool.tile([1, W64 + W32], bf16)
            nc.sync.dma_start(negio[:], ng_dram[:])
            s_row = pool.tile([4, QB + 128], f32)
            nc.scalar.dma_start(s_row[:], sr_dram[:])
            cpack = pool.tile([128, 416], f32)
            qeng = [nc.sync, nc.scalar]
            for i in range(4):
                sl = slice(i * 32, (i + 1) * 32)
                qeng[i % 2].dma_start(cpack[sl, :], cp_dram[sl, :])
            selK = cpack[:, 0:32]
            trS = cpack[0:64, 32:96]
            trH = cpack[0:C2Y, 96:128]
            id64 = cpack[0:64, 128:192]
            uvc = cpack[0:K, 192:384]
            cBsB = cpack[0:2 * K, 384:416]

            # rep operands: rows 0:NB one-hot block selector, row NB = -iota
            rep64 = pool.tile([NB + 1, W64], bf16)
            nc.vector.tensor_scalar(rep64[0:NB, :], ior64[:], 0.0, None,
                                    ALU.is_equal)
            nc.sync.dma_start(rep64[NB:NB + 1, :], negio[:, 0:W64])
            rep32 = pool.tile([NB + 1, W32], bf16)
            nc.vector.tensor_scalar(rep32[0:NB, :], ior32[:], 0.0, None,
                                    ALU.is_equal)
            nc.scalar.dma_start(rep32[NB:NB + 1, :], negio[:, W64:])

            ones1 = pool.tile([1, 1], f32)
            nc.vector.memset(ones1[:], 1.0)
            lnb1 = pool.tile([1, 1], f32)
            nc.vector.memset(lnb1[:], 1.0)

            # ---------- trig features: s_w = wsel @ s_row, then series ----
            sparts = pool.tile([128, 2], f32)
            nc.vector.memset(sparts[:], 0.0)
            cos_ins = None
            cs_ps = psum.tile([K, 2], f32, tag="pcs", bufs=1)
            for h in range(2):
                hs = slice(h * HQ, (h + 1) * HQ)
                pwt = psum.tile([128, 512], f32, tag="pA", bufs=2)
                pw = pwt[:, 0:HQ]
                nc.tensor.matmul(pw, lhsT=s_row[0:4, QB:QB + 128],
                                 rhs=s_row[0:4, hs], start=True, stop=True)
                rnd = pool.tile([128, HQ], f32, tag="rnd", bufs=2)
                nc.scalar.activation(rnd[:], pw, AF.Copy, bias=_MAGIC,
                                     scale=_INV2PI)
                kint = pool.tile([128, HQ], f32, tag="kint", bufs=2)
                nc.vector.tensor_scalar(kint[:], rnd[:], _MAGIC, None,
                                        ALU.subtract)
                sa = pool.tile([128, HQ], f32, tag="sa", bufs=2)
                nc.vector.cody_waite_cascade(sa[:], pw, kint[:],
                                             _CW1, _CW2, _CW3)
                clamp = float(np.float32(_PI))
                nc.vector.tensor_scalar(sa[:], sa[:], clamp, -clamp,
                                        ALU.min, ALU.max)
                ca = pool.tile([128, HQ], f32, tag="ca", bufs=2)
                nc.vector.add_range_wrap(ca[:], sa[:], _PI / 2, _PI, 2 * _PI)
                nc.vector.tensor_scalar(ca[:], ca[:], clamp, -clamp,
                                        ALU.min, ALU.max)
                sin_t = pool.tile([128, HQ], f32, tag="sint", bufs=2)
                nc.scalar.activation(sin_t[:], sa[:], AF.Sin,
                                     accum_out=sparts[:, 0:1])
                cos_t = pool.tile([128, HQ], f32, tag="cost", bufs=2)
                cos_ins = nc.scalar.activation(cos_t[:], ca[:], AF.Sin,
                                               accum_out=sparts[:, 1:2])
                nc.tensor.matmul(cs_ps[:], lhsT=selK, rhs=sparts[:],
                                 start=(h == 0), stop=(h == 1),
                                 skip_group_check=True)
            # switch the ACT table to Ln now; post-AR Ln finds it loaded
            lnwarm = pool.tile([1, 1], f32)
            warm_ins = nc.scalar.activation(lnwarm[:], ones1[:], AF.Ln,
                                            bias=lnb1[:])
            add_dep_helper(warm_ins.ins, cos_ins.ins, False,
                           "Ln table load after the Sin stream")
            cs_sb = pool.tile([K, 2], f32)
            nc.scalar.copy(cs_sb[:], cs_ps[:])

            # ---------- expansions (q - class into PSUM) + one-hots -------
            h1y = pool.tile([128, W64], bf16)
            rhs_y = pool.tile([128, 2 * W32], bf16)
            yx = pool.tile([128, W32], bf16)
            h1s = pool.tile([128, W32], bf16)
            h2se = pool.tile([128, W32], bf16)
            h2sw = pool.tile([128, W32], bf16)

            # q1y: 64-class, 3 chunks
            for c0, c1 in ((0, 512), (512, 1024), (1024, W64)):
                pe = psum.tile([128, 512], f32, tag="pA", bufs=2)
                nc.tensor.matmul(pe[:, 0:c1 - c0], lhsT=qT[:, 0:128],
                                 rhs=rep64[:, c0:c1], start=True, stop=True,
                                 skip_group_check=True)
                nc.vector.tensor_scalar(h1y[:, c0:c1], pe[:, 0:c1 - c0],
                                        0.0, None, ALU.is_equal)
            # q2y + shared y expansion + weighted half, 2 chunks of 320
            for c0 in (0, 320):
                c1 = c0 + 320
                pe = psum.tile([128, 512], f32, tag="pA", bufs=2)
                nc.tensor.matmul(pe[:, 0:320], lhsT=qT[:, 128:256],
                                 rhs=rep32[:, c0:c1], start=True, stop=True,
                                 skip_group_check=True)
                nc.vector.tensor_scalar(rhs_y[:, c0:c1], pe[:, 0:320],
                                        0.0, None, ALU.is_equal)
                pe2 = psum.tile([128, 512], f32, tag="pA", bufs=2)
                nc.tensor.matmul(pe2[:, 0:320], lhsT=qT[0:NB, 512:640],
                                 rhs=rep32[0:NB, c0:c1], start=True,
                                 stop=True, skip_group_check=True)
                nc.vector.tensor_scalar(yx[:, c0:c1], pe2[:, 0:320], 1.0,
                                        None, ALU.mult)
                nc.vector.tensor_tensor(rhs_y[:, W32 + c0:W32 + c1],
                                        rhs_y[:, c0:c1], yx[:, c0:c1],
                                        ALU.mult)
            # qs1 / qs2, 2 chunks each; weighted s-side on gpsimd
            for c0 in (0, 320):
                c1 = c0 + 320
                pe = psum.tile([128, 512], f32, tag="pA", bufs=2)
                nc.tensor.matmul(pe[:, 0:320], lhsT=qT[:, 256:384],
                                 rhs=rep32[:, c0:c1], start=True, stop=True,
                                 skip_group_check=True)
                nc.vector.tensor_scalar(h1s[:, c0:c1], pe[:, 0:320],
                                        0.0, None, ALU.is_equal)
                pe2 = psum.tile([128, 512], f32, tag="pA", bufs=2)
                nc.tensor.matmul(pe2[:, 0:320], lhsT=qT[:, 384:512],
                                 rhs=rep32[:, c0:c1], start=True, stop=True,
                                 skip_group_check=True)
                nc.vector.tensor_scalar(h2se[:, c0:c1], pe2[:, 0:320],
                                        0.0, None, ALU.is_equal)
                nc.gpsimd.tensor_tensor(h2sw[:, c0:c1], h2se[:, c0:c1],
                                        yx[:, c0:c1], ALU.mult)

            # ---------- weighted histograms (2 interleaved groups) --------
            psY = psum.tile([C1Y, 2 * C2Y], f32, tag="py", bufs=1)
            psSt = psum.tile([64, 96], f32, tag="pC", bufs=2)
            psS = psSt[0:C1S, 0:C2S]
            rhs_yv = rhs_y[:].rearrange("p (h x) -> p h x", h=2)
            for b in range(NB):
                nc.tensor.matmul(
                    psY[:], lhsT=h1y[:, b * C1Y:(b + 1) * C1Y],
                    rhs=rhs_yv[:, :, b * C2Y:(b + 1) * C2Y],
                    start=(b == 0), stop=(b == NB - 1),
                    skip_group_check=True)
                nc.tensor.matmul(
                    psS, lhsT=h1s[:, b * C1S:(b + 1) * C1S],
                    rhs=h2sw[:, b * C2S:(b + 1) * C2S],
                    start=(b == 0), stop=(b == NB - 1),
                    skip_group_check=True)

            # ---------- local suffix table T ----------
            hist_sb = pool.tile([C1Y, C2Y], f32)
            nc.scalar.copy(hist_sb[:], psY[:, 0:C2Y])
            ysum_sb = pool.tile([C1Y, C2Y], f32)
            nc.scalar.copy(ysum_sb[:], psY[:, C2Y:2 * C2Y])
            ys_sb = pool.tile([C1S, C2S], f32)
            nc.scalar.copy(ys_sb[:], psS)
            rowsum = pool.tile([C1Y, 1], f32)
            nc.vector.tensor_reduce(rowsum[:], psY[:, 0:C2Y], axis=X,
                                    op=ALU.add)
            htpt = psum.tile([64, 96], f32, tag="pC", bufs=2)
            htp = htpt[0:C2Y, 0:C1Y]
            nc.tensor.transpose(htp, hist_sb[:], id64)
            hts = pool.tile([C2Y, C1Y], f32)
            nc.scalar.copy(hts[:], htp)
            srfx = psum.tile([64, 96], f32, tag="pC", bufs=2)
            nc.tensor.matmul(srfx[:, 0:C2Y], lhsT=hts[:], rhs=trH,
                             start=True, stop=True, skip_group_check=True)
            nc.tensor.matmul(srfx[:, 64:65], lhsT=trS, rhs=rowsum[:],
                             start=True, stop=True, skip_group_check=True)
            t_loc = pool.tile([C1Y, C2Y], f32)
            nc.vector.tensor_scalar(t_loc[:], srfx[:, 0:C2Y],
                                    srfx[:, 64:65], None, ALU.add)

            # ---------- fused AllReduce: [T | Ysum | Ys | C/S] ----------
            PWC = C2Y + C2Y + C2S + 2          # 98 payload cols
            cc_in = dram.tile([C1Y, PWC], f32)
            cc_out = dram.tile([C1Y, PWC], f32, addr_space="Shared")
            zfill = pool.tile([C1Y - C1S, C2S + 2], f32)
            nc.vector.memset(zfill[:], 0.0)
            dbias = pool.tile([C1S, 1], f32)
            nc.vector.memset(dbias[:], float(N / 2 + 2.0))
            ibias = pool.tile([C1Y, 1], f32)
            nc.vector.memset(ibias[:], 1.5)
            parts = pool.tile([64, 3], f32)
            nc.vector.memset(parts[:], 0.0)
            ones64 = pool.tile([64, 1], f32)
            nc.vector.memset(ones64[:], 1.0)
            nc.scalar.dma_start(cc_in[C1S:C1Y, 2 * C2Y:PWC], zfill[:])
            nc.sync.dma_start(cc_in[:, 0:C2Y], t_loc[:])
            nc.sync.dma_start(cc_in[:, C2Y:2 * C2Y], ysum_sb[:])
            nc.scalar.dma_start(cc_in[0:C1S, 2 * C2Y:2 * C2Y + C2S],
                                ys_sb[:])
            nc.scalar.dma_start(cc_in[0:K, 2 * C2Y + C2S:PWC], cs_sb[:])
            nc.gpsimd.collective_compute(
                "AllReduce", ALU.add, replica_groups=groups,
                ins=[cc_in[:, :].opt()], outs=[cc_out[:, :].opt()])
            csg = pool.tile([K, 2], f32)
            nc.scalar.dma_start(csg[:], cc_out[0:K, 2 * C2Y + C2S:PWC])
            t_glob = pool.tile([C1Y, C2Y], f32)
            nc.sync.dma_start(t_glob[:], cc_out[:, 0:C2Y])
            yss = pool.tile([C1S, C2S], f32)
            nc.scalar.dma_start(yss[:], cc_out[0:C1S, 2 * C2Y:2 * C2Y + C2S])
            ysg = pool.tile([C1Y, C2Y], f32)
            nc.sync.dma_start(ysg[:], cc_out[:, C2Y:2 * C2Y])

            # ---------- dcg: series at score-bucket centers ----------
            # csg col0 = S_k, col1 = raw C_k; the TRIG_PAD correction
            # (pads contribute cos(0)=1 each) is folded into the PsAb /
            # PcAbn constant blocks: u1 = sAb*C - 480*sAb = sAb*C'.
            # luv rows 0:K = -U, K:2K = -V  (negated; Ln uses scale=-1)
            luv = pool.tile([2 * K, C1S], f32)
            u1 = pool.tile([K, C1S], f32)
            nc.vector.scalar_tensor_tensor(u1[:], uvc[:, 0:32],
                                           csg[:, 1:2], uvc[:, 128:160],
                                           ALU.mult, ALU.subtract)
            nc.vector.scalar_tensor_tensor(luv[0:K, :], uvc[:, 32:64],
                                           csg[:, 0:1], u1[:],
                                           ALU.mult, ALU.subtract)
            v1 = pool.tile([K, C1S], f32)
            nc.vector.scalar_tensor_tensor(v1[:], uvc[:, 96:128],
                                           csg[:, 1:2], uvc[:, 160:192],
                                           ALU.mult, ALU.subtract)
            nc.vector.scalar_tensor_tensor(luv[K:2 * K, :], uvc[:, 64:96],
                                           csg[:, 0:1], v1[:],
                                           ALU.mult, ALU.add)
            rank_pst = psum.tile([64, 96], f32, tag="pC", bufs=2)
            rank_ps = rank_pst[0:C1S, 0:C2S]
            nc.tensor.matmul(rank_ps, lhsT=luv[:], rhs=cBsB,
                             start=True, stop=True)
            lnds = pool.tile([C1S, C2S], f32)
            nc.scalar.activation(lnds[:], rank_ps, AF.Ln,
                                 bias=dbias[:], scale=-1.0)
            rds = pool.tile([C1S, C2S], f32)
            nc.vector.reciprocal(rds[:], lnds[:])
            scrD = pool.tile([C1S, C2S], f32)
            nc.vector.scalar_tensor_tensor(scrD[:], yss[:], LN2, rds[:],
                                           ALU.mult, ALU.mult,
                                           accum_out=parts[0:C1S, 0:1])
            # ---------- idcg: per-bucket mid-rank discount ----------
            lnis = pool.tile([C1Y, C2Y], f32)
            nc.scalar.activation(lnis[:], t_glob[:], AF.Ln, bias=ibias[:])
            ris = pool.tile([C1Y, C2Y], f32)
            nc.vector.reciprocal(ris[:], lnis[:])
            scrI = pool.tile([C1Y, C2Y], f32)
            nc.vector.scalar_tensor_tensor(scrI[:], ysg[:], LN2, ris[:],
                                           ALU.mult, ALU.mult,
                                           accum_out=parts[:, 1:2])
            nc.vector.tensor_reduce(parts[:, 2:3], ysg[:], axis=X,
                                    op=ALU.add)

            # ---------- fold partitions, final scalar loss ----------
            ps2t = psum.tile([64, 96], f32, tag="pC", bufs=2)
            ps2 = ps2t[0:1, 0:3]
            nc.tensor.matmul(ps2, lhsT=ones64[:], rhs=parts[:],
                             start=True, stop=True)
            d1 = pool.tile([1, 1], f32)
            nc.vector.tensor_scalar(d1[:], ps2t[0:1, 1:2], 1e-8, None,
                                    ALU.add)
            rec = pool.tile([1, 1], f32)
            nc.vector.reciprocal(rec[:], d1[:])
            negl = pool.tile([1, 1], f32)
            nc.vector.scalar_tensor_tensor(negl[:], ps2t[0:1, 0:1], rec[:],
                                           ones1[:], ALU.mult, ALU.subtract)
            negm = pool.tile([1, 1], f32)
            nc.vector.tensor_scalar(negm[:], ps2t[0:1, 2:3], 1.0, -1.0,
                                    ALU.is_ge, ALU.mult)
            fin = pool.tile([1, 1], f32)
            nc.vector.tensor_tensor(fin[:], negl[:], negm[:], ALU.mult)
            nc.sync.dma_start(out_dram[:], fin[:])

    nc.compile()
    return nc


def _get_nc():
    if "nc" not in _CACHE:
        _CACHE["nc"] = _build()
    return _CACHE["nc"]


def _consts():
    p = np.arange(128)
    selK = (p[:, None] // 4 == np.arange(K)[None, :]).astype(np.float32)
    a = np.arange(64)
    trS = (a[:, None] > a[None, :]).astype(np.float32)
    a2 = np.arange(C2Y)
    trH = ((a2[:, None] > a2[None, :]).astype(np.float32)
           + 0.5 * (a2[:, None] == a2[None, :]).astype(np.float32))
    id64 = np.eye(64, dtype=np.float32)
    om = _OMEGA.astype(np.float64)[:, None]
    aang = om * (LO + np.arange(C1S, dtype=np.float64)[None, :]
                 * C2S * DELTA)
    bang = om * ((np.arange(C2S, dtype=np.float64)[None, :] + 0.5) * DELTA)
    bk = _B.astype(np.float64)[:, None]
    sAb = (bk * np.sin(aang)).astype(np.float32)
    cAb = (bk * np.cos(aang)).astype(np.float32)
    # device layout: 0:32 sAb, 32:64 cAb, 64:96 -sAb, 96:128 -cAb,
    # 128:160 PAD*sAb, 160:192 -PAD*cAb  (TRIG_PAD folded into the chain:
    # u1 = sAb*C - PAD*sAb; v1 = -cAb*C - (-PAD*cAb))
    P = float(TRIG_PAD)
    uvc = np.concatenate([sAb, cAb, -sAb, -cAb, P * sAb, -P * cAb], axis=1)
    cBsB = np.concatenate([np.cos(bang), np.sin(bang)],
                          axis=0).astype(np.float32)
    cpack = np.zeros((128, 416), np.float32)
    cpack[:, 0:32] = selK
    cpack[0:64, 32:96] = trS
    cpack[0:C2Y, 96:128] = trH
    cpack[0:64, 128:192] = id64
    cpack[0:K, 192:384] = uvc
    cpack[0:2 * K, 384:416] = cBsB
    negio = np.concatenate([
        -np.tile(np.arange(C1Y, dtype=np.float32), NB),
        -np.tile(np.arange(C2Y, dtype=np.float32), NB)]).reshape(1, -1)
    return np.ascontiguousarray(cpack), \
        np.ascontiguousarray(negio).astype(ml_dtypes.bfloat16)


def _in_maps(logits, targets):
    s = np.asarray(logits, dtype=np.float32).reshape(-1)
    y = np.asarray(targets, dtype=np.float32).reshape(-1)
    npad = NCORES * PB
    s_pad = np.zeros((npad,), np.float32)
    s_pad[:N] = s
    y_pad = np.zeros((npad,), np.float32)
    y_pad[:N] = y
    q = np.clip(np.floor(y.astype(np.float64) * QSY).astype(np.int64),
                0, QSY - 1)
    qy1_pad = np.full((npad,), -1.0, np.float32)
    qy1_pad[:N] = (q // C2Y).astype(np.float32)
    qy2_pad = np.full((npad,), -1.0, np.float32)
    qy2_pad[:N] = (q % C2Y).astype(np.float32)
    qs = np.clip(np.floor((s.astype(np.float64) - LO) / DELTA).astype(
        np.int64), 0, MBS - 1)
    qs1_pad = np.full((npad,), -1.0, np.float32)
    qs1_pad[:N] = (qs // C2S).astype(np.float32)
    qs2_pad = np.full((npad,), -1.0, np.float32)
    qs2_pad[:N] = (qs % C2S).astype(np.float32)
    cpack, negio = _consts()
    pp = np.arange(128)
    wsel = (_OMEGA[pp // 4][None, :]
            * (np.arange(4)[:, None] == pp[None, :] % 4)).astype(np.float32)
    maps = []
    for d in range(NCORES):
        sl = slice(d * PB, (d + 1) * PB)
        sv = s_pad[sl]
        s_row2 = np.concatenate([sv.reshape(4, QB), wsel],
                                axis=1).astype(np.float32)
        # per-block lhsT layout [NB+1, 5*128]: qT[b, g*128+p] = value of
        # item b*128+p in group g; row NB = ones
        qT = np.ones((NB + 1, 5 * 128), np.float32)
        qT[0:NB, 0:128] = qy1_pad[sl].reshape(NB, 128)
        qT[0:NB, 128:256] = qy2_pad[sl].reshape(NB, 128)
        qT[0:NB, 256:384] = qs1_pad[sl].reshape(NB, 128)
        qT[0:NB, 384:512] = qs2_pad[sl].reshape(NB, 128)
        qT[0:NB, 512:640] = y_pad[sl].reshape(NB, 128)
        maps.append({
            "qT": np.ascontiguousarray(qT).astype(ml_dtypes.bfloat16),
            "negio": negio,
            "s_row2": np.ascontiguousarray(s_row2),
            "cpack": cpack,
        })
    return maps


def kernel(logits, targets):
    nc = _get_nc()
    res = run_bass_kernel_spmd(nc, _in_maps(logits, targets),
                               core_ids=list(range(NCORES)))
    out = np.asarray(res.results[0]["out"], dtype=np.float32)
    return out.reshape(())
